# revision 1
# baseline (speedup 1.0000x reference)
"""Trainium2 Bass kernel for nn_BidirectionalGRU (B=8,S=1024,D=1024).

Pipeline: rmsnorm -> 2x bidirectional GRU -> out-proj + residual -> rmsnorm
-> SwiGLU FFN + residual.

All matmuls run in float32r (fp32 data, reduced-precision multiply, 1 cyc/row).
GRU scan: h.T kept as PE stationary [128,8] per K-tile, w_hh.T streamed from
SBUF; 4 PE column groups (tile_position=(0,32j)) produce a gate-grouped PSUM
layout (partition 32j+row; 768 cols = r|z|n 256-col slices of group j, where
group j owns gate/h slices [256j:256(j+1)]).  h.T is rebuilt each step with 2
PE transposes.  Biases/norm-scale fold into GEMM epilogues; every f32r matmul
is structured to carry at most one fresh semaphore wait (walrus S3_LW limit):
accumulation groups open with a K=1 zero-matmul.
"""
import contextlib
import numpy as np

import concourse.bacc as bacc
import concourse.tile as tile
from concourse import mybir
from concourse.bass import ds
from concourse.bass_utils import run_bass_kernel_spmd
from concourse.masks import make_identity

F32 = mybir.dt.float32
F32R = mybir.dt.float32r
BF16 = mybir.dt.bfloat16
AF = mybir.ActivationFunctionType
ALU = mybir.AluOpType

B, S, D, H3, G, FFN = 8, 1024, 1024, 3072, 4, 2816
NT = (B * S) // 128          # 64 token tiles (token = b*S + t)
KD = D // 128                # 8
KF = FFN // 128              # 22
EPS = 1e-5
NP = 104                     # partitions spanned by grouped layout (3*32+8)


# ================================================================ host prep
def gate_perm():
    idx = []
    for j in range(G):
        for blk in range(3):
            base = blk * 1024 + j * 256
            idx.extend(range(base, base + 256))
    return np.array(idx)

PERM = gate_perm()


def prep_scan_weights(w_hh_d):
    """[3072,1024] -> [128, KD*3072]: w[p, k*H3 + n] = w_hh_perm[n, 128k+p]."""
    wp = w_hh_d[PERM]
    wt = wp.T.reshape(KD, 128, H3).transpose(1, 0, 2)
    return np.ascontiguousarray(wt.reshape(128, KD * H3), dtype=np.float32)


def prep_gemm_weights(w_ih_d, norm_w=None):
    wp = w_ih_d[PERM]
    if norm_w is not None:
        wp = wp * norm_w[None, :]
    return np.ascontiguousarray(wp.T, dtype=np.float32)


def prep_gemm_bias(b_ih_d, b_hh_d):
    """[128,3072] broadcast: rz cols get b_ih+b_hh, n cols b_ih only."""
    bi = b_ih_d[PERM].copy()
    bh = b_hh_d[PERM]
    m = np.where(np.arange(H3) % 768 < 512, bh, 0.0)
    b = (bi + m).astype(np.float32)
    return np.ascontiguousarray(np.broadcast_to(b, (128, H3)), dtype=np.float32)


def prep_bhn_scan(b_hh_d):
    bh = b_hh_d[PERM].reshape(G, 3, 256)[:, 2, :]
    out = np.zeros((128, 256), np.float32)
    for j in range(G):
        out[32 * j:32 * j + 32, :] = bh[j][None, :]
    return out


# ============================================================ device builders
def build_norm_stats(tc, x_nat, s_sb):
    nc = tc.nc
    with tc.tile_pool(name="nstat", bufs=3) as pool:
        for i in range(NT):
            xt = pool.tile([128, D], F32, name="xt")
            nc.sync.dma_start(xt[:], x_nat[i * 128:(i + 1) * 128, :])
            sq = pool.tile([128, D], F32, name="sq")
            ss = pool.tile([128, 1], F32, name="ss")
            nc.scalar.activation(sq[:], xt[:], AF.Square, accum_out=ss[:])
            m = pool.tile([128, 1], F32, name="m")
            nc.vector.tensor_scalar(m[:], ss[:], 1.0 / D, EPS,
                                    op0=ALU.mult, op1=ALU.add)
            r = pool.tile([128, 1], F32, name="r")
            nc.vector.reciprocal(r[:], m[:])
            nc.scalar.activation(s_sb[:, i:i + 1], r[:], AF.Sqrt)


def build_xg_gemm(tc, ctx, stat_view, n_k, w, bias, s_sb, out_v,
                  zeros_st, zrhs, wdt=F32R):
    """out[token, g, 768c] = s*(x @ w) + bias for one direction.

    stat_view: [n_k*128, B*S] f32r AP (xT, or concat hT view) - stationary.
    w: [n_k*128, 3072] f32r; bias [128, 3072] f32; out_v: [B*S, G, 768] f32.
    """
    nc = tc.nc
    with contextlib.ExitStack() as c:
        wp = c.enter_context(tc.tile_pool(name="xg_w", bufs=1))
        pool = c.enter_context(tc.tile_pool(name="xg_t", bufs=3))
        stp = c.enter_context(tc.tile_pool(name="xg_s", bufs=2))
        pp = c.enter_context(tc.tile_pool(name="xg_p", bufs=4, space="PSUM"))

        bias_sb = wp.tile([128, H3], F32, name="bias_sb")
        nc.sync.dma_start(bias_sb[:], bias[:, :])
        U = 8
        for c0 in range(0, H3, 512):
            # resident w slices for this chunk
            wc = pool.tile([128, n_k * 512], wdt, name="wc")
            for k in range(n_k):
                nc.sync.dma_start(wc[:, k * 512:(k + 1) * 512],
                                  w[k * 128:(k + 1) * 128, c0:c0 + 512])
            with tc.For_i(0, NT // U) as iv:
                for u in range(U):
                    tv = iv * U + u
                    tok = tv * 128
                    sts = []
                    for k in range(n_k):
                        stt = stp.tile([128, 128], wdt, name=f"st{k}")
                        if isinstance(stat_view, tuple):
                            sv = stat_view[k // KD]
                            kk = k % KD
                        else:
                            sv, kk = stat_view, k
                        nc.sync.dma_start(
                            stt[:], sv[kk * 128:(kk + 1) * 128, ds(tok, 128)])
                        sts.append(stt)
                    ps = pp.tile([128, 512], F32, name="ps")
                    nc.tensor.matmul(ps[:], zeros_st[:], zrhs[:],
                                     start=True, stop=False)
                    for k in range(n_k):
                        nc.tensor.matmul(ps[:], sts[k][:],
                                         wc[:, k * 512:(k + 1) * 512],
                                         start=False, stop=(k == n_k - 1))
                    o = pool.tile([128, 512], F32, name="o")
                    if s_sb is not None:
                        nc.vector.scalar_tensor_tensor(
                            o[:], ps[:], s_sb[:, ds(tv, 1)],
                            bias_sb[:, c0:c0 + 512],
                            op0=ALU.mult, op1=ALU.add)
                    else:
                        nc.vector.tensor_add(o[:], ps[:],
                                             bias_sb[:, c0:c0 + 512])
                    # cols c0..c0+512 within group g0 (c0 multiple of 512;
                    # 768-group boundary: split writes
                    cc = c0
                    while cc < c0 + 512:
                        g, gc = divmod(cc, 768)
                        take = min(768 - gc, c0 + 512 - cc)
                        nc.sync.dma_start(
                            out_v[ds(tok, 128), g, gc:gc + take],
                            o[:, cc - c0:cc - c0 + take])
                        cc += take


def build_scan(tc, w_src, bhn_src, xg_v, hT_out, reverse, zeros_st, zrhs,
               ident, zeros_in=None, s_len=S):
    """One GRU direction over S steps, all B rows.

    xg_v: [B*S, G, 768] f32; hT_out: [D, B, S] f32r.
    """
    nc = tc.nc
    with contextlib.ExitStack() as c:
        wp = c.enter_context(tc.tile_pool(name="sc_w", bufs=1))
        st = c.enter_context(tc.tile_pool(name="sc_s", bufs=1))
        pool = c.enter_context(tc.tile_pool(name="sc_t", bufs=3))
        pp = c.enter_context(tc.tile_pool(name="sc_p", bufs=2, space="PSUM"))
        ppt = c.enter_context(tc.tile_pool(name="sc_pt", bufs=2,
                                           space="PSUM"))

        w_sb = wp.tile([128, KD * H3], BF16, name="w_sb")
        nc.sync.dma_start(w_sb[:], w_src[:, :])
        bhn = wp.tile([128, 256], F32, name="bhn")
        nc.sync.dma_start(bhn[:], bhn_src[:, :])

        hgrp = st.tile([128, 256], F32, name="hgrp")
        nc.gpsimd.memset(hgrp[:], 0.0)
        # h.T history: slot u holds compact cols [c*32 + j*8 + r] (64/step)
        U = 16
        hT_hist = st.tile([128, U * 64], BF16, name="hT_hist")
        nc.sync.dma_start(hT_hist[:], zeros_in[:, 0:U * 64])  # bf16 zeros

        # xg viewed [t, g, b, c] for per-step fetch
        xg_t = xg_v.rearrange("(b t) g c -> t g b c", b=B)

        with tc.For_i(0, s_len // U) as iv:
            for u in range(U):
                if reverse:
                    t_el = iv * (-U) + (s_len - 1 - u)
                else:
                    t_el = iv * U + u
                scan_step(tc, pool, pp, ppt, w_sb, bhn, hgrp, hT_hist,
                          u, (u - 1) % U, xg_t, t_el, zeros_st, zrhs, ident)
            # flush h.T for these U steps to HBM: hT_out [D, B, S].
            # K-tile k covers hT_out rows [128k, 128(k+1)) (d = 128k+p);
            # compact col in hist = (k%2)*32 + (k//2)*8 + b.
            # Per-(k,b) DMAs: 2 real dims + 1 symbolic (3-dim DMA AP limit).
            hist3 = hT_hist.rearrange("p (s x) -> p s x", s=U)
            for k in range(KD):
                base = (k % 2) * 32 + (k // 2) * 8
                for b in range(B):
                    src = hist3[:, :, base + b]        # [p, slot]
                    if reverse:
                        # slot s holds t = (s_len-1-iv*U) - s
                        dst = hT_out[k * 128:(k + 1) * 128, b,
                                     ds(iv * (-U) + (s_len - U), U)]
                        src = src[:, ::-1]
                    else:
                        dst = hT_out[k * 128:(k + 1) * 128, b,
                                     ds(iv * U, U)]
                    nc.sync.dma_start(dst, src)


def scan_step(tc, pool, pp, ppt, w_sb, bhn, hgrp, hT_hist, slot, pslot,
              xg_t, t_el, zeros_st, zrhs, ident):
    nc = tc.nc
    xgt = pool.tile([128, 768], F32, name="xgt")
    for j in range(G):
        srcj = xg_t[ds(t_el, 1), j, :, :].rearrange("a b c -> (a b) c")
        nc.sync.dma_start(xgt[32 * j:32 * j + B, :], srcj)

    gates = pp.tile([128, 768], F32, name="gates")
    nc.tensor.matmul(gates[:, 0:512], zeros_st[:], zrhs[:],
                     start=True, stop=False)
    nc.tensor.matmul(gates[:, 512:768], zeros_st[:], zrhs[:, 0:256],
                     start=True, stop=False)
    for k in range(KD):
        j2, c2 = divmod(k, 2)
        lof = pslot * 64 + c2 * 32 + j2 * 8
        lhsT = hT_hist[:, lof:lof + 8]
        for j in range(G):
            wof = k * H3 + j * 768
            nc.tensor.matmul(gates[32 * j:32 * j + 8, 0:512], lhsT,
                             w_sb[:, wof:wof + 512], start=False, stop=False,
                             tile_position=(0, 32 * j))
            nc.tensor.matmul(gates[32 * j:32 * j + 8, 512:768], lhsT,
                             w_sb[:, wof + 512:wof + 768], start=False,
                             stop=(k == KD - 1), tile_position=(0, 32 * j))

    grz = pool.tile([128, 512], F32, name="grz")
    nc.vector.tensor_add(grz[:NP], gates[:NP, 0:512], xgt[:NP, 0:512])
    rz = pool.tile([128, 512], F32, name="rz")
    nc.scalar.activation(rz[:NP], grz[:NP], AF.Sigmoid)
    t2a = pool.tile([128, 256], F32, name="t2a")
    nc.vector.tensor_add(t2a[:NP], gates[:NP, 512:768], bhn[:NP])
    t2 = pool.tile([128, 256], F32, name="t2")
    nc.vector.tensor_mul(t2[:NP], rz[:NP, 0:256], t2a[:NP])
    npre = pool.tile([128, 256], F32, name="npre")
    nc.vector.tensor_add(npre[:NP], t2[:NP], xgt[:NP, 512:768])
    nn = pool.tile([128, 256], F32, name="nn")
    nc.scalar.activation(nn[:NP], npre[:NP], AF.Tanh)
    dlt = pool.tile([128, 256], F32, name="dlt")
    nc.vector.tensor_sub(dlt[:NP], hgrp[:NP], nn[:NP])
    e = pool.tile([128, 256], F32, name="e")
    nc.vector.tensor_mul(e[:NP], rz[:NP, 256:512], dlt[:NP])
    nc.vector.tensor_add(hgrp[:NP], nn[:NP], e[:NP])

    tp = ppt.tile([128, 256], F32, name="tp")
    for cc in range(2):
        nc.tensor.transpose(tp[:, 128 * cc:128 * cc + NP],
                            hgrp[0:NP, 128 * cc:128 * (cc + 1)],
                            ident[0:NP, 0:NP])
    # compact copy PSUM -> hT_hist slot: col c*32 + j*8 + r  <- tp col
    # 128c + 32j + r (r<8)
    tp4 = tp.rearrange("p (c j r) -> p c j r", c=2, j=G)[:, :, :, 0:B]
    ho = hT_hist[:, slot * 64:(slot + 1) * 64]
    ho4 = ho.rearrange("p (c j r) -> p c j r", c=2, j=G)
    nc.scalar.activation(ho4, tp4, AF.Copy)


def build_proj(tc, dram, zeros_st, zrhs, ident):
    """F-A: x2 = x + concat1 @ gru_out.T; s2; x2nT -> HBM."""
    nc = tc.nc
    h1f = dram["hT1_f"].rearrange("d b s -> d (b s)")
    h1b = dram["hT1_b"].rearrange("d b s -> d (b s)")
    with contextlib.ExitStack() as c:
        wp = c.enter_context(tc.tile_pool(name="pj_w", bufs=1))
        pool = c.enter_context(tc.tile_pool(name="pj_t", bufs=3))
        stp = c.enter_context(tc.tile_pool(name="pj_s", bufs=2))
        pp = c.enter_context(tc.tile_pool(name="pj_p", bufs=4, space="PSUM"))

        gw = wp.tile([128, 2 * KD * D], BF16, name="gw")
        for k in range(2 * KD):
            nc.sync.dma_start(gw[:, k * D:(k + 1) * D],
                              dram["gru_wT"][k * 128:(k + 1) * 128, :])

        U = 4
        with tc.For_i(0, NT // U) as iv:
            for u in range(U):
                tv = iv * U + u
                tok = tv * 128
                sts = []
                for k in range(2 * KD):
                    stt = stp.tile([128, 128], BF16, name=f"pst{k}")
                    srcv = h1f if k < KD else h1b
                    kk = k % KD
                    nc.sync.dma_start(
                        stt[:], srcv[kk * 128:(kk + 1) * 128, ds(tok, 128)])
                    sts.append(stt)
                x2 = pool.tile([128, D], F32, name="x2")
                for cc in range(2):
                    ps = pp.tile([128, 512], F32, name="ps")
                    nc.tensor.matmul(ps[:], zeros_st[:], zrhs[:],
                                     start=True, stop=False)
                    for k in range(2 * KD):
                        nc.tensor.matmul(
                            ps[:], sts[k][:],
                            gw[:, k * D + 512 * cc:k * D + 512 * cc + 512],
                            start=False, stop=(k == 2 * KD - 1))
                    xt = pool.tile([128, 512], F32, name="xt")
                    nc.sync.dma_start(
                        xt[:], dram["x_nat"][ds(tok, 128),
                                             512 * cc:512 * cc + 512])
                    nc.vector.tensor_add(x2[:, 512 * cc:512 * cc + 512],
                                         ps[:], xt[:])
                nc.sync.dma_start(dram["x2"][ds(tok, 128), :], x2[:])
                # rms scale
                sq = pool.tile([128, D], F32, name="sq")
                ssum = pool.tile([128, 1], F32, name="ssum")
                nc.scalar.activation(sq[:], x2[:], AF.Square,
                                     accum_out=ssum[:])
                m = pool.tile([128, 1], F32, name="m")
                nc.vector.tensor_scalar(m[:], ssum[:], 1.0 / D, EPS,
                                        op0=ALU.mult, op1=ALU.add)
                r = pool.tile([128, 1], F32, name="r")
                nc.vector.reciprocal(r[:], m[:])
                s2 = pool.tile([128, 1], F32, name="s2")
                nc.scalar.activation(s2[:], r[:], AF.Sqrt)
                x2n = pool.tile([128, D], F32, name="x2n")
                nc.vector.tensor_scalar_mul(x2n[:], x2[:], s2[:])
                for k in range(KD):
                    tpp = pp.tile([128, 128], F32, name="tpp")
                    nc.tensor.transpose(tpp[:], x2n[:, k * 128:(k + 1) * 128],
                                        ident[:])
                    xc = pool.tile([128, 128], F32R, name="xc")
                    nc.scalar.activation(xc[:], tpp[:], AF.Copy)
                    nc.sync.dma_start(
                        dram["x2nT"][k * 128:(k + 1) * 128, ds(tok, 128)],
                        xc[:])


def build_ffn13(tc, dram, zeros_st, zrhs, ident):
    """F-B: h1 = silu(x2n@w1.T)*(x2n@w3.T); h1T -> HBM."""
    nc = tc.nc
    with contextlib.ExitStack() as c:
        wp = c.enter_context(tc.tile_pool(name="fb_w", bufs=1))
        pool = c.enter_context(tc.tile_pool(name="fb_t", bufs=3))
        stp = c.enter_context(tc.tile_pool(name="fb_s", bufs=2))
        pp = c.enter_context(tc.tile_pool(name="fb_p", bufs=2, space="PSUM"))

        w1 = wp.tile([128, KD * FFN], F32R, name="w1")
        w3 = wp.tile([128, KD * FFN], F32R, name="w3")
        for k in range(KD):
            nc.sync.dma_start(w1[:, k * FFN:(k + 1) * FFN],
                              dram["w1T"][k * 128:(k + 1) * 128, :])
            nc.sync.dma_start(w3[:, k * FFN:(k + 1) * FFN],
                              dram["w3T"][k * 128:(k + 1) * 128, :])

        FCH = [(c0, min(512, FFN - c0)) for c0 in range(0, FFN, 512)]
        with tc.For_i(0, NT) as tv:
            tok = tv * 128
            sts = []
            for k in range(KD):
                stt = stp.tile([128, 128], F32R, name=f"bst{k}")
                nc.sync.dma_start(
                    stt[:], dram["x2nT"][k * 128:(k + 1) * 128, ds(tok, 128)])
                sts.append(stt)
            for (c0, cn) in FCH:
                p1 = pp.tile([128, 512], F32, name="p1")
                p3 = pp.tile([128, 512], F32, name="p3")
                nc.tensor.matmul(p1[:, :cn], zeros_st[:], zrhs[:, :cn],
                                 start=True, stop=False)
                nc.tensor.matmul(p3[:, :cn], zeros_st[:], zrhs[:, :cn],
                                 start=True, stop=False)
                for k in range(KD):
                    nc.tensor.matmul(p1[:, :cn], sts[k][:],
                                     w1[:, k * FFN + c0:k * FFN + c0 + cn],
                                     start=False, stop=(k == KD - 1))
                    nc.tensor.matmul(p3[:, :cn], sts[k][:],
                                     w3[:, k * FFN + c0:k * FFN + c0 + cn],
                                     start=False, stop=(k == KD - 1))
                sl = pool.tile([128, 512], F32, name="sl")
                nc.scalar.activation(sl[:, :cn], p1[:, :cn], AF.Silu)
                h1c = pool.tile([128, 512], F32, name="h1c")
                nc.vector.tensor_mul(h1c[:, :cn], sl[:, :cn], p3[:, :cn])
                # transpose 128-col blocks -> h1T
                for q in range(cn // 128):
                    tpp = pp.tile([128, 128], F32, name="tpp")
                    nc.tensor.transpose(
                        tpp[:], h1c[:, q * 128:(q + 1) * 128], ident[:])
                    hc = pool.tile([128, 128], F32R, name="hc")
                    nc.scalar.activation(hc[:], tpp[:], AF.Copy)
                    kf = (c0 + q * 128) // 128
                    nc.sync.dma_start(
                        dram["h1T"][kf * 128:(kf + 1) * 128, ds(tok, 128)],
                        hc[:])


def build_ffn2(tc, dram, zeros_st, zrhs):
    """F-C: y = x2 + h1 @ w2.T."""
    nc = tc.nc
    with contextlib.ExitStack() as c:
        wp = c.enter_context(tc.tile_pool(name="fc_w", bufs=1))
        pool = c.enter_context(tc.tile_pool(name="fc_t", bufs=3))
        stp = c.enter_context(tc.tile_pool(name="fc_s", bufs=2))
        pp = c.enter_context(tc.tile_pool(name="fc_p", bufs=4, space="PSUM"))

        w2 = wp.tile([128, KF * D], F32R, name="w2")
        for k in range(KF):
            nc.sync.dma_start(w2[:, k * D:(k + 1) * D],
                              dram["w2T"][k * 128:(k + 1) * 128, :])

        U = 2
        with tc.For_i(0, NT // U) as iv:
            for u in range(U):
                tv = iv * U + u
                tok = tv * 128
                sts = []
                for k in range(KF):
                    stt = stp.tile([128, 128], F32R, name=f"cst{k}")
                    nc.sync.dma_start(
                        stt[:],
                        dram["h1T"][k * 128:(k + 1) * 128, ds(tok, 128)])
                    sts.append(stt)
                for cc in range(2):
                    ps = pp.tile([128, 512], F32, name="ps")
                    nc.tensor.matmul(ps[:], zeros_st[:], zrhs[:],
                                     start=True, stop=False)
                    for k in range(KF):
                        nc.tensor.matmul(
                            ps[:], sts[k][:],
                            w2[:, k * D + 512 * cc:k * D + 512 * cc + 512],
                            start=False, stop=(k == KF - 1))
                    xt = pool.tile([128, 512], F32, name="xt")
                    nc.sync.dma_start(
                        xt[:], dram["x2"][ds(tok, 128),
                                          512 * cc:512 * cc + 512])
                    yo = pool.tile([128, 512], F32, name="yo")
                    nc.vector.tensor_add(yo[:], ps[:], xt[:])
                    nc.sync.dma_start(
                        dram["y"][ds(tok, 128), 512 * cc:512 * cc + 512],
                        yo[:])


def build_program(nc, debug=False):
    dram = {}

    def din(name, shape, dt=F32R):
        dram[name] = nc.dram_tensor(name, shape, dt, kind="ExternalInput").ap()

    def dout(name, shape, dt=F32):
        dram[name] = nc.dram_tensor(name, shape, dt,
                                    kind="ExternalOutput").ap()

    def dtmp(name, shape, dt=F32R):
        dram[name] = nc.dram_tensor(name, shape, dt).ap()

    din("x_nat", [B * S, D], F32)
    din("xT", [D, B * S])
    for dd in ("f", "b"):
        din(f"wA_{dd}", [D, H3])
        din(f"biasA_{dd}", [128, H3], F32)
        din(f"wD_{dd}", [2 * D, H3], BF16)
        din(f"biasD_{dd}", [128, H3], F32)
        for L in (0, 1):
            din(f"wS{L}_{dd}", [128, KD * H3], BF16)
            din(f"bhn{L}_{dd}", [128, 256], F32)
    din("zeros", [128, 1024])
    din("zeros_bf", [128, 1024], BF16)
    din("gru_wT", [2 * D, D], BF16)
    din("w1T", [D, FFN])
    din("w3T", [D, FFN])
    din("w2T", [FFN, D])
    dout("y", [B * S, D])

    for dd in ("f", "b"):
        dtmp(f"xg_{dd}", [B * S, G, 768], F32)
        dtmp(f"hT0_{dd}", [D, B, S], BF16)
        dtmp(f"hT1_{dd}", [D, B, S], BF16)
    dtmp("x2", [B * S, D], F32)
    dtmp("x2nT", [D, B * S])
    dtmp("h1T", [FFN, B * S])

    with tile.TileContext(nc) as tc:
        with tc.tile_pool(name="consts", bufs=1) as consts:
            zeros_st = consts.tile([1, 128], F32R, name="zeros_st")
            nc.sync.dma_start(zeros_st[:], dram["zeros"][0:1, 0:128])
            zrhs = consts.tile([1, 512], F32R, name="zrhs")
            nc.sync.dma_start(zrhs[:], dram["zeros"][0:1, 0:512])
            ident = consts.tile([128, 128], F32, name="ident")
            make_identity(nc, ident[:])
            s_sb = consts.tile([128, NT], F32, name="s_sb")

            build_norm_stats(tc, dram["x_nat"], s_sb)
            for dd in ("f", "b"):
                build_xg_gemm(tc, None, dram["xT"], KD, dram[f"wA_{dd}"],
                              dram[f"biasA_{dd}"], s_sb, dram[f"xg_{dd}"],
                              zeros_st, zrhs)
            for dd, rev in (("f", False), ("b", True)):
                build_scan(tc, dram[f"wS0_{dd}"], dram[f"bhn0_{dd}"],
                           dram[f"xg_{dd}"], dram[f"hT0_{dd}"], rev,
                           zeros_st, zrhs, ident, dram["zeros_bf"])
            import os as _os
            _lim = _os.environ.get("KPHASES", "")
            h0f = dram["hT0_f"].rearrange("d b s -> d (b s)")
            h0b = dram["hT0_b"].rearrange("d b s -> d (b s)")
            concat0 = (h0f, h0b)
            if _lim != "A":
                for dd, rev in (("f", False), ("b", True)):
                    build_xg_gemm(tc, None, concat0, 2 * KD,
                                  dram[f"wD_{dd}"], dram[f"biasD_{dd}"],
                                  None, dram[f"xg_{dd}"],
                                  zeros_st, zrhs, wdt=BF16)
                    build_scan(tc, dram[f"wS1_{dd}"], dram[f"bhn1_{dd}"],
                               dram[f"xg_{dd}"], dram[f"hT1_{dd}"], rev,
                               zeros_st, zrhs, ident, dram["zeros_bf"])
                build_proj(tc, dram, zeros_st, zrhs, ident)
                build_ffn13(tc, dram, zeros_st, zrhs, ident)
                build_ffn2(tc, dram, zeros_st, zrhs)
            if debug:
                for nm, shp, dt in (("xg_f", [B * S, G * 768], F32),
                                    ("xg_b", [B * S, G * 768], F32),
                                    ("hT0_f", [D, B * S], BF16),
                                    ("hT0_b", [D, B * S], BF16),
                                    ("x2", [B * S, D], F32)):
                    dbg = nc.dram_tensor("dbg_" + nm, shp, dt,
                                         kind="ExternalOutput").ap()
                    srcv = dram[nm]
                    flat = srcv.rearrange("a b c -> a (b c)") if len(
                        srcv.shape) == 3 else srcv
                    nc.sync.dma_start(dbg[:, :], flat[:, :])
    return dram


# ================================================================== driver
_CACHE = {}


def _host_inputs(inputs):
    import ml_dtypes
    bf = ml_dtypes.bfloat16
    x = np.asarray(inputs["x"], np.float32)
    gnw = np.asarray(inputs["gru_norm_w"], np.float32)
    fnw = np.asarray(inputs["ffn_norm_w"], np.float32)
    im = {}
    x_nat = np.ascontiguousarray(x.reshape(B * S, D))
    im["x_nat"] = x_nat
    im["xT"] = np.ascontiguousarray(x_nat.T)
    for di, dd in ((0, "f"), (1, "b")):
        im[f"wA_{dd}"] = prep_gemm_weights(
            np.asarray(inputs["w_ih_l0"], np.float32)[di], gnw)
        im[f"biasA_{dd}"] = prep_gemm_bias(
            np.asarray(inputs["b_ih_l0"], np.float32)[di],
            np.asarray(inputs["b_hh_l0"], np.float32)[di])
        im[f"wD_{dd}"] = prep_gemm_weights(
            np.asarray(inputs["w_ih_l1"], np.float32)[di]).astype(bf)
        im[f"biasD_{dd}"] = prep_gemm_bias(
            np.asarray(inputs["b_ih_l1"], np.float32)[di],
            np.asarray(inputs["b_hh_l1"], np.float32)[di])
        for L in (0, 1):
            im[f"wS{L}_{dd}"] = prep_scan_weights(
                np.asarray(inputs[f"w_hh_l{L}"], np.float32)[di]).astype(bf)
            im[f"bhn{L}_{dd}"] = prep_bhn_scan(
                np.asarray(inputs[f"b_hh_l{L}"], np.float32)[di])
    im["zeros"] = np.zeros((128, 1024), np.float32)
    im["zeros_bf"] = np.zeros((128, 1024), bf)
    im["gru_wT"] = np.ascontiguousarray(
        np.asarray(inputs["gru_out_w"], np.float32).T).astype(bf)
    im["w1T"] = np.ascontiguousarray(
        (np.asarray(inputs["w1"], np.float32) * fnw[None, :]).T)
    im["w3T"] = np.ascontiguousarray(
        (np.asarray(inputs["w3"], np.float32) * fnw[None, :]).T)
    im["w2T"] = np.ascontiguousarray(np.asarray(inputs["w2"], np.float32).T)
    return im


def get_compiled(n_cores=8):
    if "nc" not in _CACHE:
        import os
        nc = bacc.Bacc("TRN2", target_bir_lowering=False, debug=False,
                       num_devices=n_cores)
        build_program(nc, debug=bool(os.environ.get("KDEBUG")))
        nc.compile()
        _CACHE["nc"] = nc
        _CACHE["n_cores"] = n_cores
    return _CACHE["nc"], _CACHE["n_cores"]


def kernel(**inputs) -> np.ndarray:
    im = _host_inputs(inputs)
    nc, n_cores = get_compiled()
    in_maps = [im for _ in range(n_cores)]
    res = run_bass_kernel_spmd(nc, in_maps, core_ids=list(range(n_cores)))
    return res.results[0]["y"].reshape(B, S, D)



# revision 76
# speedup vs baseline: 1.0994x; 1.0994x over previous
"""Trainium2 Bass kernel for nn_BidirectionalGRU (B=8,S=1024,D=1024), 8-core.

Pipeline: rmsnorm -> 2x bidirectional GRU -> out-proj + residual -> rmsnorm
-> SwiGLU FFN + residual.

8-core SPMD split:
  core c: dir d = c%2 (0=fwd, 1=bwd), pair q = c//2.
  Every core holds x pre-permuted into ITS scan order (bwd cores get
  time-reversed x from the host), so the device program is identical on
  all cores; only input data differs (plus partition_id-derived offsets).

  P0  norm stats + xg0 GEMM for my 256 time-steps x 8 batch rows
      -> AllGather over my direction group [[0,2,4,6],[1,3,5,7]]
  P1  L0 scan over all 1024 steps (my direction only)
      flush h.T in both scan order (slot d) and reversed (slot 1-d)
      -> pairwise AllToAll [[0,1],...]: out = [h_f | h_b] in MY t-order
  P2  xg1 GEMM for my 256 steps from concat h0 -> AllGather
  P3  L1 scan (my direction) -> flush -> AllToAll
  P4  proj + rmsnorm + SwiGLU FFN for my 128 steps (x_loc rows [0:1024])
  Host reassembles y slices (un-reversing bwd cores').

Scan inner loop: h.T stationary [128,8] per K-tile, w_hh.T streamed from
SBUF; 4 PE column groups (tile_position=(0,32j)) produce a gate-grouped
PSUM layout (partition 32j+row; 768 cols = r|z|n 256-col slices of group
j).  h.T rebuilt each step with 2 PE transposes.  Every f32r matmul
carries at most one fresh semaphore wait; accumulation groups open with
a K=1 zero-matmul.
"""
import contextlib
import numpy as np

import concourse.bacc as bacc
import concourse.tile as tile
from concourse import mybir
from concourse.bass import ds
from concourse.bass_utils import run_bass_kernel_spmd
from concourse.masks import make_identity

F32 = mybir.dt.float32
F32R = mybir.dt.float32r
BF16 = mybir.dt.bfloat16
AF = mybir.ActivationFunctionType
ALU = mybir.AluOpType

B, S, D, H3, G, FFN = 8, 1024, 1024, 3072, 4, 2816
KD = D // 128                # 8
KF = FFN // 128              # 22
EPS = 1e-5
NP = 104                     # partitions spanned by grouped layout (3*32+8)
NCORE = 8
SLOC = S // 4                # 256 time-steps owned per core (xg phases)
NTL = (SLOC * B) // 128      # 16 local token tiles
SFFN = SLOC // 2             # 128 time-steps in FFN phase
NTF = (SFFN * B) // 128      # 8 ffn token tiles
D2 = 2 * D

AG_GROUPS = [[0, 2, 4, 6], [1, 3, 5, 7]]
A2A_GROUPS = [[0, 1], [2, 3], [4, 5], [6, 7]]

# When True, hardware For_i loops are python-unrolled so TimelineSim's
# no_exec mode can schedule the program (it cannot resolve loop branches).
SIM_UNROLL = False

# V2 scan: gate layout [rA zA nA rB zB nB] per group; xg and b_hh_n are
# folded into PSUM by identity/ones opener matmuls; the two hidden halves
# are processed as separate DVE/ACT chains and the h.T transposes of step
# t-1 are interleaved into step t's matmul stream (software pipelining).
import os as _os
V2_SCAN = _os.environ.get("KV2", "1") != "0"


def run_loop(tc, n, body):
    if SIM_UNROLL:
        for i in range(n):
            body(i)
    else:
        with tc.For_i(0, n) as iv:
            body(iv)


# ================================================================ host prep
def gate_perm():
    idx = []
    for j in range(G):
        for blk in range(3):
            base = blk * 1024 + j * 256
            idx.extend(range(base, base + 256))
    return np.array(idx)


def gate_perm2():
    idx = []
    for j in range(G):
        for h in range(2):
            for blk in range(3):
                base = blk * 1024 + j * 256 + h * 128
                idx.extend(range(base, base + 128))
    return np.array(idx)

PERM = gate_perm2() if V2_SCAN else gate_perm()
# rz columns within each 768-col group: v1 [0:512]; v2 two [.. 256] runs
RZ_MASK = (np.arange(H3) % 384 < 256) if V2_SCAN else \
          (np.arange(H3) % 768 < 512)


def prep_scan_weights(w_hh_d):
    """[3072,1024] -> [128, KD*3072]: w[p, k*H3 + n] = w_hh_perm[n, 128k+p]."""
    wp = w_hh_d[PERM]
    wt = wp.T.reshape(KD, 128, H3).transpose(1, 0, 2)
    return np.ascontiguousarray(wt.reshape(128, KD * H3), dtype=np.float32)


def prep_gemm_weights(w_ih_d, norm_w=None):
    wp = w_ih_d[PERM]
    if norm_w is not None:
        wp = wp * norm_w[None, :]
    return np.ascontiguousarray(wp.T, dtype=np.float32)


def prep_gemm_bias(b_ih_d, b_hh_d):
    """[128,3072] broadcast: rz cols get b_ih+b_hh, n cols b_ih only."""
    bi = b_ih_d[PERM].copy()
    bh = b_hh_d[PERM]
    m = np.where(RZ_MASK, bh, 0.0)
    b = (bi + m).astype(np.float32)
    return np.ascontiguousarray(np.broadcast_to(b, (128, H3)), dtype=np.float32)


def prep_ident8():
    """[128,128] masked identity: diag 1 at rows 32j+r (r<8), else 0."""
    a = np.zeros((128, 128), np.float32)
    for j in range(G):
        for r in range(8):
            a[32 * j + r, 32 * j + r] = 1.0
    return a


def prep_ones8():
    """[128,128] band broadcast: row 32j -> out cols 32j..32j+32."""
    a = np.zeros((128, 128), np.float32)
    for j in range(G):
        a[32 * j, 32 * j:32 * j + 32] = 1.0
    return a


def prep_bhn_scan(b_hh_d):
    bp = b_hh_d[PERM]
    if V2_SCAN:   # group block [rA zA nA rB zB nB] -> [nA nB]
        b6 = bp.reshape(G, 6, 128)
        bh = np.concatenate([b6[:, 2, :], b6[:, 5, :]], axis=-1)
    else:
        bh = bp.reshape(G, 3, 256)[:, 2, :]
    out = np.zeros((128, 256), np.float32)
    for j in range(G):
        out[32 * j:32 * j + 32, :] = bh[j][None, :]
    return out


# ============================================================ device builders
def build_norm_stats(tc, x_nat, s_sb, nt):
    nc = tc.nc
    with tc.tile_pool(name="nstat", bufs=3) as pool:
        for i in range(nt):
            xt = pool.tile([128, D], F32, name="xt")
            nc.sync.dma_start(xt[:], x_nat[i * 128:(i + 1) * 128, :])
            sq = pool.tile([128, D], F32, name="sq")
            ss = pool.tile([128, 1], F32, name="ss")
            nc.scalar.activation(sq[:], xt[:], AF.Square, accum_out=ss[:])
            m = pool.tile([128, 1], F32, name="m")
            nc.vector.tensor_scalar(m[:], ss[:], 1.0 / D, EPS,
                                    op0=ALU.mult, op1=ALU.add)
            r = pool.tile([128, 1], F32, name="r")
            nc.vector.reciprocal(r[:], m[:])
            nc.scalar.activation(s_sb[:, i:i + 1], r[:], AF.Sqrt)


def build_xg_gemm(tc, fetch_st, n_k, nt, w, bias, s_sb, out_v,
                  zeros_st, zrhs, wdt=F32R, U=8):
    """out[token, g, 768c] = s*(x @ w) + bias, bf16 out.

    fetch_st(stp, k, tv) -> stationary tile [128,128] for K-tile k,
    token-tile tv (tv is a register expression).
    out_v: [nt*128, G, 768] BF16 view.
    """
    nc = tc.nc
    with contextlib.ExitStack() as c:
        wp = c.enter_context(tc.tile_pool(name="xg_w", bufs=1))
        pool = c.enter_context(tc.tile_pool(name="xg_t", bufs=3))
        stp = c.enter_context(tc.tile_pool(name="xg_s", bufs=2))
        pp = c.enter_context(tc.tile_pool(name="xg_p", bufs=4, space="PSUM"))

        bias_sb = wp.tile([128, H3], F32, name="bias_sb")
        nc.sync.dma_start(bias_sb[:], bias[:, :])
        for c0 in range(0, H3, 512):
            # resident w slices for this chunk
            wc = pool.tile([128, n_k * 512], wdt, name="wc")
            for k in range(n_k):
                nc.sync.dma_start(wc[:, k * 512:(k + 1) * 512],
                                  w[k * 128:(k + 1) * 128, c0:c0 + 512])
            def chunk_body(iv, c0=c0, wc=wc):
                for u in range(U):
                    tv = iv * U + u
                    tok = tv * 128
                    sts = [fetch_st(stp, k, tv) for k in range(n_k)]
                    ps = pp.tile([128, 512], F32, name="ps")
                    nc.tensor.matmul(ps[:], zeros_st[:], zrhs[:],
                                     start=True, stop=False)
                    for k in range(n_k):
                        nc.tensor.matmul(ps[:], sts[k][:],
                                         wc[:, k * 512:(k + 1) * 512],
                                         start=False, stop=(k == n_k - 1))
                    o = pool.tile([128, 512], BF16, name="o")
                    if s_sb is not None:
                        nc.vector.scalar_tensor_tensor(
                            o[:], ps[:], s_sb[:, ds(tv, 1)],
                            bias_sb[:, c0:c0 + 512],
                            op0=ALU.mult, op1=ALU.add)
                    else:
                        nc.vector.tensor_add(o[:], ps[:],
                                             bias_sb[:, c0:c0 + 512])
                    # cols c0..c0+512 within group g0 (c0 multiple of 512;
                    # 768-group boundary: split writes)
                    cc = c0
                    while cc < c0 + 512:
                        g, gc = divmod(cc, 768)
                        take = min(768 - gc, c0 + 512 - cc)
                        nc.sync.dma_start(
                            out_v[ds(tok, 128), g, gc:gc + take],
                            o[:, cc - c0:cc - c0 + take])
                        cc += take

            run_loop(tc, nt // U, chunk_body)


def build_scan(tc, w_src, bhn_src, xg_v, hT_loc, hT_rev,
               zeros_st, zrhs, ident, zeros_bf, st2=None):
    """One GRU direction over S steps in local scan order, all B rows.

    xg_v: [S*B, G, 768] bf16 (rows t'*B + b); hT_loc: [D, S*B] bf16
    (col = t'*B + b) written in scan order; hT_rev: same shape written
    time-reversed (the AllGather payload for the pair partner).
    """
    nc = tc.nc
    with contextlib.ExitStack() as c:
        wp = c.enter_context(tc.tile_pool(name="sc_w", bufs=1))
        st = c.enter_context(tc.tile_pool(name="sc_s", bufs=1))
        pool = c.enter_context(tc.tile_pool(name="sc_t", bufs=3))
        pp = c.enter_context(tc.tile_pool(name="sc_p", bufs=2, space="PSUM"))
        ppt = c.enter_context(tc.tile_pool(name="sc_pt", bufs=2,
                                           space="PSUM"))

        w_sb = wp.tile([128, KD * H3], BF16, name="w_sb")
        nc.sync.dma_start(w_sb[:], w_src[:, :])
        bhn = wp.tile([128, 256], BF16 if V2_SCAN else F32, name="bhn")
        nc.sync.dma_start(bhn[:], bhn_src[:, :])

        hgrp = st.tile([128, 256], F32, name="hgrp")
        nc.gpsimd.memset(hgrp[:], 0.0)
        xgt_pp = None
        if V2_SCAN:
            xgt_pp = [st.tile([128, 768], BF16, name=f"xgtp{i}")
                      for i in range(2)]
            for t in xgt_pp:
                nc.sync.dma_start(t[:], zeros_bf[:, 0:768])
        # h.T history: slot u holds compact cols [c*32 + j*8 + r] (64/step)
        U = 16
        hT_hist = st.tile([128, U * 64], BF16, name="hT_hist")
        nc.sync.dma_start(hT_hist[:], zeros_bf[:, 0:U * 64])  # bf16 zeros

        # xg viewed [t, g, b, c] for per-step fetch
        xg_t = xg_v.rearrange("(t b) g c -> t g b c", b=B)

        def scan_body(iv):
            pend = []
            for u in range(U):
                t_el = iv * U + u
                if V2_SCAN:
                    gates, xgt = scan_step2_mm(
                        tc, pool, pp, st2, hT_hist, w_sb, bhn, xg_t, t_el,
                        u, (u - 1) % U, zeros_st, zrhs, pend,
                        xgt_pp[u % 2])
                    pend = [
                        scan_step2_chain(tc, pool, ppt, gates, xgt, hgrp,
                                         hT_hist, u, half, ident)
                        for half in (0, 1)]
                else:
                    scan_step(tc, pool, pp, ppt, w_sb, bhn, hgrp, hT_hist,
                              u, (u - 1) % U, xg_t, t_el, zeros_st, zrhs,
                              ident)
            if V2_SCAN:
                pend[0]()
                pend[1]()
            # flush h.T for these U steps to HBM, both orders.
            # K-tile k covers hT rows [128k, 128(k+1)) (d = 128k+p);
            # compact col in hist = (k%2)*32 + (k//2)*8 + b.
            hist3 = hT_hist.rearrange("p (s x) -> p s x", s=U)
            hl3 = hT_loc.rearrange("d (s b) -> d s b", b=B)
            hr3 = hT_rev.rearrange("d (s b) -> d s b", b=B)
            for k in range(KD):
                base = (k % 2) * 32 + (k // 2) * 8
                for b in range(B):
                    src = hist3[:, :, base + b]        # [p, slot]
                    nc.sync.dma_start(
                        hl3[k * 128:(k + 1) * 128, ds(iv * U, U), b],
                        src)
                    nc.sync.dma_start(
                        hr3[k * 128:(k + 1) * 128,
                            ds(iv * (-U) + (S - U), U), b],
                        src[:, ::-1])

        run_loop(tc, S // U, scan_body)


def scan_step2_mm(tc, pool, pp, st2, hT_hist, w_sb, bhn, xg_t, t_el,
                  slot, pslot, zeros_st, zrhs, pend, xgt):
    """V2: emit step-t matmuls with step-(t-1) transposes interleaved.

    Gate cols per group j: [rA zA nA rB zB nB] (128 each).  xg and b_hh_n
    enter PSUM via opener matmuls (identity / ones lhsT on the diagonal
    32x32 tile).  Returns the gates PSUM tile + xgt tile.
    """
    nc = tc.nc
    ident8, ones8 = st2[0], st2[1]
    for j in range(G):
        srcj = xg_t[ds(t_el, 1), j, :, :].rearrange("a b c -> (a b) c")
        nc.sync.dma_start(xgt[32 * j:32 * j + B, :], srcj)

    if pend:
        pend[0]()          # trA(t-1) + copyA(t-1)
    # PSUM layout: half-A at cols [0:384] (bank 0), half-B at [512:896]
    # (bank 1) -- a matmul output may not cross a 512-col PSUM bank.
    gates = pp.tile([128, 1024], F32, name="gates")
    # Openers (full 128-partition K=128 masked-identity matmuls): write
    # xg into the rz columns and b_hh_n into the n columns, opening each
    # bank's accumulation group at partition offset 0.
    nc.tensor.matmul(gates[:, 0:256], ident8[:], xgt[:, 0:256],
                     start=True, stop=False)
    nc.tensor.matmul(gates[:, 256:384], ones8[:], bhn[:, 0:128],
                     start=False, stop=False)
    nc.tensor.matmul(gates[:, 512:768], ident8[:], xgt[:, 384:640],
                     start=True, stop=False)
    nc.tensor.matmul(gates[:, 768:896], ones8[:], bhn[:, 128:256],
                     start=False, stop=False)

    def mm_pass(half, kpar):
        for k in range(kpar, KD, 2):
            j2, c2 = divmod(k, 2)
            lof = pslot * 64 + c2 * 32 + j2 * 8
            lhsT = hT_hist[:, lof:lof + 8]
            for j in range(G):
                wof = k * H3 + j * 768 + 384 * half
                nc.tensor.matmul(
                    gates[32 * j:32 * j + 8, 512 * half:512 * half + 384],
                    lhsT, w_sb[:, wof:wof + 384], start=False, stop=False,
                    tile_position=(0, 32 * j))

    mm_pass(0, 0)          # half-A cols, k even (needs copyA(t-1))
    if pend:
        pend[1]()          # trB(t-1) + copyB(t-1)
    mm_pass(0, 1)          # half-A cols, k odd (needs copyB(t-1))
    mm_pass(1, 0)
    mm_pass(1, 1)
    # zero-accumulate closers: stop=True over all 128 partitions closes
    # each bank's accumulation group (M=32 openers opened 32 rows/band,
    # M=8 gate matmuls could only close 8).
    nc.tensor.matmul(gates[:, 0:8], zeros_st[:], zrhs[:, 0:8],
                     start=False, stop=True)
    nc.tensor.matmul(gates[:, 512:520], zeros_st[:], zrhs[:, 0:8],
                     start=False, stop=True)
    return gates, xgt


def scan_step2_chain(tc, pool, ppt, gates, xgt, hgrp, hT_hist, slot, half,
                     ident):
    """V2 per-half DVE/ACT chain; returns the deferred transpose+copy."""
    nc = tc.nc
    hof = 512 * half           # PSUM column offset of this half
    xof = 384 * half           # xgt (SBUF) column offset
    hhof = 128 * half
    rz = pool.tile([128, 256], F32, name=f"rz{half}")
    nc.scalar.activation(rz[:NP], gates[:NP, hof:hof + 256], AF.Sigmoid)
    t2 = pool.tile([128, 128], F32, name=f"t2{half}")
    nc.vector.tensor_mul(t2[:NP], rz[:NP, 0:128],
                         gates[:NP, hof + 256:hof + 384])
    npre = pool.tile([128, 128], F32, name=f"npre{half}")
    nc.vector.tensor_add(npre[:NP], t2[:NP], xgt[:NP, xof + 256:xof + 384])
    nn = pool.tile([128, 128], F32, name=f"nn{half}")
    nc.scalar.activation(nn[:NP], npre[:NP], AF.Tanh)
    dlt = pool.tile([128, 128], F32, name=f"dlt{half}")
    nc.vector.tensor_sub(dlt[:NP], hgrp[:NP, hhof:hhof + 128], nn[:NP])
    e = pool.tile([128, 128], F32, name=f"e{half}")
    nc.vector.tensor_mul(e[:NP], rz[:NP, 128:256], dlt[:NP])
    nc.vector.tensor_add(hgrp[:NP, hhof:hhof + 128], nn[:NP], e[:NP])

    def do_tr():
        tp = ppt.tile([128, 128], F32, name=f"tp{half}")
        nc.tensor.transpose(tp[:, 0:NP], hgrp[0:NP, hhof:hhof + 128],
                            ident[0:NP, 0:NP])
        tp3 = tp.rearrange("p (j r) -> p j r", j=G)[:, :, 0:B]
        ho = hT_hist[:, slot * 64 + half * 32:slot * 64 + half * 32 + 32]
        ho3 = ho.rearrange("p (j r) -> p j r", j=G)
        nc.scalar.activation(ho3, tp3, AF.Copy)

    return do_tr


def scan_step(tc, pool, pp, ppt, w_sb, bhn, hgrp, hT_hist, slot, pslot,
              xg_t, t_el, zeros_st, zrhs, ident):
    nc = tc.nc
    xgt = pool.tile([128, 768], BF16, name="xgt")
    for j in range(G):
        srcj = xg_t[ds(t_el, 1), j, :, :].rearrange("a b c -> (a b) c")
        nc.sync.dma_start(xgt[32 * j:32 * j + B, :], srcj)

    gates = pp.tile([128, 768], F32, name="gates")
    nc.tensor.matmul(gates[:, 0:512], zeros_st[:], zrhs[:],
                     start=True, stop=False)
    nc.tensor.matmul(gates[:, 512:768], zeros_st[:], zrhs[:, 0:256],
                     start=True, stop=False)
    for k in range(KD):
        j2, c2 = divmod(k, 2)
        lof = pslot * 64 + c2 * 32 + j2 * 8
        lhsT = hT_hist[:, lof:lof + 8]
        for j in range(G):
            wof = k * H3 + j * 768
            nc.tensor.matmul(gates[32 * j:32 * j + 8, 0:512], lhsT,
                             w_sb[:, wof:wof + 512], start=False, stop=False,
                             tile_position=(0, 32 * j))
            nc.tensor.matmul(gates[32 * j:32 * j + 8, 512:768], lhsT,
                             w_sb[:, wof + 512:wof + 768], start=False,
                             stop=(k == KD - 1), tile_position=(0, 32 * j))

    grz = pool.tile([128, 512], F32, name="grz")
    nc.vector.tensor_add(grz[:NP], gates[:NP, 0:512], xgt[:NP, 0:512])
    rz = pool.tile([128, 512], F32, name="rz")
    nc.scalar.activation(rz[:NP], grz[:NP], AF.Sigmoid)
    t2a = pool.tile([128, 256], F32, name="t2a")
    nc.vector.tensor_add(t2a[:NP], gates[:NP, 512:768], bhn[:NP])
    t2 = pool.tile([128, 256], F32, name="t2")
    nc.vector.tensor_mul(t2[:NP], rz[:NP, 0:256], t2a[:NP])
    npre = pool.tile([128, 256], F32, name="npre")
    nc.vector.tensor_add(npre[:NP], t2[:NP], xgt[:NP, 512:768])
    nn = pool.tile([128, 256], F32, name="nn")
    nc.scalar.activation(nn[:NP], npre[:NP], AF.Tanh)
    dlt = pool.tile([128, 256], F32, name="dlt")
    nc.vector.tensor_sub(dlt[:NP], hgrp[:NP], nn[:NP])
    e = pool.tile([128, 256], F32, name="e")
    nc.vector.tensor_mul(e[:NP], rz[:NP, 256:512], dlt[:NP])
    nc.vector.tensor_add(hgrp[:NP], nn[:NP], e[:NP])

    tp = ppt.tile([128, 256], F32, name="tp")
    for cc in range(2):
        nc.tensor.transpose(tp[:, 128 * cc:128 * cc + NP],
                            hgrp[0:NP, 128 * cc:128 * (cc + 1)],
                            ident[0:NP, 0:NP])
    # compact copy PSUM -> hT_hist slot: col c*32 + j*8 + r  <- tp col
    # 128c + 32j + r (r<8)
    tp4 = tp.rearrange("p (c j r) -> p c j r", c=2, j=G)[:, :, :, 0:B]
    ho = hT_hist[:, slot * 64:(slot + 1) * 64]
    ho4 = ho.rearrange("p (c j r) -> p c j r", c=2, j=G)
    nc.scalar.activation(ho4, tp4, AF.Copy)


def build_proj_ffn(tc, dram, r_tcol, rd_peer_D, zeros_st, zrhs, ident):
    """P4: x2 = x + concat1 @ gru_out.T; rms; SwiGLU FFN; y."""
    nc = tc.nc
    h1l = dram["hTloc1"]
    h1p = dram["agh1_out"]
    with contextlib.ExitStack() as c:
        wp = c.enter_context(tc.tile_pool(name="pj_w", bufs=1))
        pool = c.enter_context(tc.tile_pool(name="pj_t", bufs=3))
        stp = c.enter_context(tc.tile_pool(name="pj_s", bufs=2))
        pp = c.enter_context(tc.tile_pool(name="pj_p", bufs=4, space="PSUM"))

        gw = wp.tile([128, 2 * KD * D], BF16, name="gw")
        for k in range(2 * KD):
            nc.sync.dma_start(gw[:, k * D:(k + 1) * D],
                              dram["gru_wT"][k * 128:(k + 1) * 128, :])

        def proj_body(tv):
            tok = tv * 128
            sts = []
            for k in range(2 * KD):
                stt = stp.tile([128, 128], BF16, name=f"pst{k}")
                if k < KD:
                    src = h1l[k * 128:(k + 1) * 128,
                              ds(r_tcol + tv * 128, 128)]
                else:
                    src = h1p[ds(rd_peer_D + (k - KD) * 128, 128),
                              ds(r_tcol + tv * 128, 128)]
                nc.sync.dma_start(stt[:], src)
                sts.append(stt)
            x2 = pool.tile([128, D], F32, name="x2")
            for cc in range(2):
                ps = pp.tile([128, 512], F32, name="ps")
                nc.tensor.matmul(ps[:], zeros_st[:], zrhs[:],
                                 start=True, stop=False)
                for k in range(2 * KD):
                    nc.tensor.matmul(
                        ps[:], sts[k][:],
                        gw[:, k * D + 512 * cc:k * D + 512 * cc + 512],
                        start=False, stop=(k == 2 * KD - 1))
                xt = pool.tile([128, 512], F32, name="xt")
                nc.sync.dma_start(
                    xt[:], dram["x_loc"][ds(tok, 128),
                                         512 * cc:512 * cc + 512])
                nc.vector.tensor_add(x2[:, 512 * cc:512 * cc + 512],
                                     ps[:], xt[:])
            nc.sync.dma_start(dram["x2"][ds(tok, 128), :], x2[:])
            # rms scale
            sq = pool.tile([128, D], F32, name="sq")
            ssum = pool.tile([128, 1], F32, name="ssum")
            nc.scalar.activation(sq[:], x2[:], AF.Square,
                                 accum_out=ssum[:])
            m = pool.tile([128, 1], F32, name="m")
            nc.vector.tensor_scalar(m[:], ssum[:], 1.0 / D, EPS,
                                    op0=ALU.mult, op1=ALU.add)
            r = pool.tile([128, 1], F32, name="r")
            nc.vector.reciprocal(r[:], m[:])
            s2 = pool.tile([128, 1], F32, name="s2")
            nc.scalar.activation(s2[:], r[:], AF.Sqrt)
            x2n = pool.tile([128, D], F32, name="x2n")
            nc.vector.tensor_scalar_mul(x2n[:], x2[:], s2[:])
            for k in range(KD):
                tpp = pp.tile([128, 128], F32, name="tpp")
                nc.tensor.transpose(tpp[:], x2n[:, k * 128:(k + 1) * 128],
                                    ident[:])
                xc = pool.tile([128, 128], F32R, name="xc")
                nc.scalar.activation(xc[:], tpp[:], AF.Copy)
                nc.sync.dma_start(
                    dram["x2nT"][k * 128:(k + 1) * 128, ds(tok, 128)],
                    xc[:])

        run_loop(tc, NTF, proj_body)


def build_ffn13(tc, dram, zeros_st, zrhs, ident):
    """h1 = silu(x2n@w1.T)*(x2n@w3.T); h1T -> HBM."""
    nc = tc.nc
    with contextlib.ExitStack() as c:
        wp = c.enter_context(tc.tile_pool(name="fb_w", bufs=1))
        pool = c.enter_context(tc.tile_pool(name="fb_t", bufs=3))
        stp = c.enter_context(tc.tile_pool(name="fb_s", bufs=2))
        pp = c.enter_context(tc.tile_pool(name="fb_p", bufs=2, space="PSUM"))

        w1 = wp.tile([128, KD * FFN], F32R, name="w1")
        w3 = wp.tile([128, KD * FFN], F32R, name="w3")
        for k in range(KD):
            nc.sync.dma_start(w1[:, k * FFN:(k + 1) * FFN],
                              dram["w1T"][k * 128:(k + 1) * 128, :])
            nc.sync.dma_start(w3[:, k * FFN:(k + 1) * FFN],
                              dram["w3T"][k * 128:(k + 1) * 128, :])

        FCH = [(c0, min(512, FFN - c0)) for c0 in range(0, FFN, 512)]

        def f13_body(tv):
            tok = tv * 128
            sts = []
            for k in range(KD):
                stt = stp.tile([128, 128], F32R, name=f"bst{k}")
                nc.sync.dma_start(
                    stt[:], dram["x2nT"][k * 128:(k + 1) * 128, ds(tok, 128)])
                sts.append(stt)
            for (c0, cn) in FCH:
                p1 = pp.tile([128, 512], F32, name="p1")
                p3 = pp.tile([128, 512], F32, name="p3")
                nc.tensor.matmul(p1[:, :cn], zeros_st[:], zrhs[:, :cn],
                                 start=True, stop=False)
                nc.tensor.matmul(p3[:, :cn], zeros_st[:], zrhs[:, :cn],
                                 start=True, stop=False)
                for k in range(KD):
                    nc.tensor.matmul(p1[:, :cn], sts[k][:],
                                     w1[:, k * FFN + c0:k * FFN + c0 + cn],
                                     start=False, stop=(k == KD - 1))
                    nc.tensor.matmul(p3[:, :cn], sts[k][:],
                                     w3[:, k * FFN + c0:k * FFN + c0 + cn],
                                     start=False, stop=(k == KD - 1))
                sl = pool.tile([128, 512], F32, name="sl")
                nc.scalar.activation(sl[:, :cn], p1[:, :cn], AF.Silu)
                h1c = pool.tile([128, 512], F32, name="h1c")
                nc.vector.tensor_mul(h1c[:, :cn], sl[:, :cn], p3[:, :cn])
                # transpose 128-col blocks -> h1T
                for q in range(cn // 128):
                    tpp = pp.tile([128, 128], F32, name="tpp")
                    nc.tensor.transpose(
                        tpp[:], h1c[:, q * 128:(q + 1) * 128], ident[:])
                    hc = pool.tile([128, 128], F32R, name="hc")
                    nc.scalar.activation(hc[:], tpp[:], AF.Copy)
                    kf = (c0 + q * 128) // 128
                    nc.sync.dma_start(
                        dram["h1T"][kf * 128:(kf + 1) * 128, ds(tok, 128)],
                        hc[:])

        run_loop(tc, NTF, f13_body)


def build_ffn2(tc, dram, zeros_st, zrhs):
    """y = x2 + h1 @ w2.T."""
    nc = tc.nc
    with contextlib.ExitStack() as c:
        wp = c.enter_context(tc.tile_pool(name="fc_w", bufs=1))
        pool = c.enter_context(tc.tile_pool(name="fc_t", bufs=3))
        stp = c.enter_context(tc.tile_pool(name="fc_s", bufs=2))
        pp = c.enter_context(tc.tile_pool(name="fc_p", bufs=4, space="PSUM"))

        w2 = wp.tile([128, KF * D], F32R, name="w2")
        for k in range(KF):
            nc.sync.dma_start(w2[:, k * D:(k + 1) * D],
                              dram["w2T"][k * 128:(k + 1) * 128, :])

        def f2_body(tv):
            tok = tv * 128
            sts = []
            for k in range(KF):
                stt = stp.tile([128, 128], F32R, name=f"cst{k}")
                nc.sync.dma_start(
                    stt[:],
                    dram["h1T"][k * 128:(k + 1) * 128, ds(tok, 128)])
                sts.append(stt)
            for cc in range(2):
                ps = pp.tile([128, 512], F32, name="ps")
                nc.tensor.matmul(ps[:], zeros_st[:], zrhs[:],
                                 start=True, stop=False)
                for k in range(KF):
                    nc.tensor.matmul(
                        ps[:], sts[k][:],
                        w2[:, k * D + 512 * cc:k * D + 512 * cc + 512],
                        start=False, stop=(k == KF - 1))
                xt = pool.tile([128, 512], F32, name="xt")
                nc.sync.dma_start(
                    xt[:], dram["x2"][ds(tok, 128),
                                      512 * cc:512 * cc + 512])
                yo = pool.tile([128, 512], F32, name="yo")
                nc.vector.tensor_add(yo[:], ps[:], xt[:])
                nc.sync.dma_start(
                    dram["y"][ds(tok, 128), 512 * cc:512 * cc + 512],
                    yo[:])

        run_loop(tc, NTF, f2_body)


def build_program(nc):
    dram = {}

    def din(name, shape, dt=F32R):
        dram[name] = nc.dram_tensor(name, shape, dt, kind="ExternalInput").ap()

    def dout(name, shape, dt=F32):
        dram[name] = nc.dram_tensor(name, shape, dt,
                                    kind="ExternalOutput").ap()

    def dtmp(name, shape, dt):
        dram[name] = nc.dram_tensor(name, shape, dt).ap()

    din("x_loc", [SLOC * B, D], F32)
    din("x_locT", [D, SLOC * B])
    din("wA", [D, H3])
    din("biasA", [128, H3], F32)
    din("wD", [D2, H3], BF16)
    din("biasD", [128, H3], F32)
    for L in (0, 1):
        din(f"wS{L}", [128, KD * H3], BF16)
        din(f"bhn{L}", [128, 256], BF16 if V2_SCAN else F32)
    if V2_SCAN:
        din("ident8", [128, 128], BF16)
        din("ones8", [128, 128], BF16)
    din("zeros", [128, 1024])
    din("zeros_bf", [128, 1024], BF16)
    din("meta", [1, 2], mybir.dt.uint32)
    din("gru_wT", [D2, D], BF16)
    din("w1T", [D, FFN])
    din("w3T", [D, FFN])
    din("w2T", [FFN, D])
    dout("y", [SFFN * B, D])

    for L in (0, 1):
        dtmp(f"ag{L}_in", [SLOC * B, G, 768], BF16)
        dtmp(f"ag{L}_out", [S * B, G, 768], BF16)
        dtmp(f"hTloc{L}", [D, S * B], BF16)
        dtmp(f"agh{L}_in", [D, S * B], BF16)
        dtmp(f"agh{L}_out", [D2, S * B], BF16)
    dtmp("x2", [SFFN * B, D], F32)
    dtmp("x2nT", [D, SFFN * B], F32R)
    dtmp("h1T", [FFN, SFFN * B], F32R)

    with tile.TileContext(nc) as tc:
        # per-core offsets from the meta input (nc.partition_id() breaks
        # execution in this environment): [0] = peer_slot * D (row offset
        # of the pair partner's shard in agh*_out), [1] = r_trow * B (col
        # offset of my t'-rows in the [D, S*B] h.T layout).
        t0 = nc.alloc_registers(f"meta_hp_{nc.next_id()}", mybir.ALL_ENGINES)
        nc.regs_load(t0, dram["meta"][0:1, 0:1])
        rd_peer_D = nc.snap(t0, donate=True, min_val=0, max_val=D)
        t1 = nc.alloc_registers(f"meta_tc_{nc.next_id()}", mybir.ALL_ENGINES)
        nc.regs_load(t1, dram["meta"][0:1, 1:2])
        r_tcol = nc.snap(t1, donate=True, min_val=0,
                         max_val=(S - SLOC) * B)

        with tc.tile_pool(name="consts", bufs=1) as consts:
            zeros_st = consts.tile([1, 128], F32R, name="zeros_st")
            nc.sync.dma_start(zeros_st[:], dram["zeros"][0:1, 0:128])
            zrhs = consts.tile([1, 512], F32R, name="zrhs")
            nc.sync.dma_start(zrhs[:], dram["zeros"][0:1, 0:512])
            ident = consts.tile([128, 128], F32, name="ident")
            make_identity(nc, ident[:])
            s_sb = consts.tile([128, NTL], F32, name="s_sb")
            st2 = None
            if V2_SCAN:
                i8 = consts.tile([128, 128], BF16, name="i8")
                nc.sync.dma_start(i8[:], dram["ident8"][:, :])
                o8 = consts.tile([128, 128], BF16, name="o8")
                nc.sync.dma_start(o8[:], dram["ones8"][:, :])
                st2 = (i8, o8)

            with nc.named_scope("P0_xg0"):
                build_norm_stats(tc, dram["x_loc"], s_sb, NTL)

                def fetch_x(stp, k, tv):
                    stt = stp.tile([128, 128], F32R, name=f"st{k}")
                    nc.sync.dma_start(
                        stt[:],
                        dram["x_locT"][k * 128:(k + 1) * 128, ds(tv * 128, 128)])
                    return stt

                build_xg_gemm(tc, fetch_x, KD, NTL, dram["wA"],
                              dram["biasA"], s_sb, dram["ag0_in"],
                              zeros_st, zrhs)
                nc.gpsimd.collective_compute(
                    "AllGather", ALU.bypass, replica_groups=AG_GROUPS,
                    ins=[dram["ag0_in"]], outs=[dram["ag0_out"]])

            with nc.named_scope("P1_scan0"):
                build_scan(tc, dram["wS0"], dram["bhn0"],
                           dram["ag0_out"],
                           dram["hTloc0"], dram["agh0_in"],
                           zeros_st, zrhs, ident, dram["zeros_bf"], st2)
                nc.gpsimd.collective_compute(
                    "AllGather", ALU.bypass, replica_groups=A2A_GROUPS,
                    ins=[dram["agh0_in"]], outs=[dram["agh0_out"]])

            with nc.named_scope("P2_xg1"):
                h0l = dram["hTloc0"]
                h0p = dram["agh0_out"]

                def fetch_h0(stp, k, tv):
                    stt = stp.tile([128, 128], BF16, name=f"st{k}")
                    if k < KD:
                        src = h0l[k * 128:(k + 1) * 128,
                                  ds(r_tcol + tv * 128, 128)]
                    else:
                        src = h0p[ds(rd_peer_D + (k - KD) * 128, 128),
                                  ds(r_tcol + tv * 128, 128)]
                    nc.sync.dma_start(stt[:], src)
                    return stt

                build_xg_gemm(tc, fetch_h0, 2 * KD, NTL, dram["wD"],
                              dram["biasD"], None, dram["ag1_in"],
                              zeros_st, zrhs, wdt=BF16)
                nc.gpsimd.collective_compute(
                    "AllGather", ALU.bypass, replica_groups=AG_GROUPS,
                    ins=[dram["ag1_in"]], outs=[dram["ag1_out"]])

            with nc.named_scope("P3_scan1"):
                build_scan(tc, dram["wS1"], dram["bhn1"],
                           dram["ag1_out"],
                           dram["hTloc1"], dram["agh1_in"],
                           zeros_st, zrhs, ident, dram["zeros_bf"], st2)
                nc.gpsimd.collective_compute(
                    "AllGather", ALU.bypass, replica_groups=A2A_GROUPS,
                    ins=[dram["agh1_in"]], outs=[dram["agh1_out"]])

            with nc.named_scope("P4_ffn"):
                build_proj_ffn(tc, dram, r_tcol, rd_peer_D,
                               zeros_st, zrhs, ident)
                build_ffn13(tc, dram, zeros_st, zrhs, ident)
                build_ffn2(tc, dram, zeros_st, zrhs)
            if _os.environ.get("KDEBUG"):
                for nm, shp, dt in (
                        ("ag0_in", [SLOC * B, G * 768], BF16),
                        ("hTloc0", [D, S * B], BF16),
                        ("agh0_out", [D2, S * B], BF16),
                        ("ag1_in", [SLOC * B, G * 768], BF16),
                        ("hTloc1", [D, S * B], BF16),
                        ("x2", [SFFN * B, D], F32)):
                    dbg = nc.dram_tensor("dbg_" + nm, shp, dt,
                                         kind="ExternalOutput").ap()
                    srcv = dram[nm]
                    flat = srcv.rearrange("a b c -> a (b c)") if len(
                        srcv.shape) == 3 else srcv
                    nc.sync.dma_start(dbg[:, :], flat[:, :])
    return dram


# ================================================================== driver
_CACHE = {}


def _host_inputs(inputs):
    """Build the 8 per-core input maps."""
    import ml_dtypes
    bf = ml_dtypes.bfloat16
    x = np.asarray(inputs["x"], np.float32)
    gnw = np.asarray(inputs["gru_norm_w"], np.float32)
    fnw = np.asarray(inputs["ffn_norm_w"], np.float32)

    zeros = np.zeros((128, 1024), np.float32)
    zeros_bf = np.zeros((128, 1024), bf)
    gru_wT_nat = np.ascontiguousarray(
        np.asarray(inputs["gru_out_w"], np.float32).T).astype(bf)
    w1T = np.ascontiguousarray(
        (np.asarray(inputs["w1"], np.float32) * fnw[None, :]).T)
    w3T = np.ascontiguousarray(
        (np.asarray(inputs["w3"], np.float32) * fnw[None, :]).T)
    w2T = np.ascontiguousarray(np.asarray(inputs["w2"], np.float32).T)

    def dir_blocks(w, d):
        """Reorder rows [f(0:D); b(D:2D)] -> [dir d block; dir 1-d block]."""
        if d == 0:
            return w
        return np.ascontiguousarray(np.concatenate([w[D:2 * D], w[0:D]]))

    per_dir = {}
    for d in (0, 1):
        wD_nat = prep_gemm_weights(
            np.asarray(inputs["w_ih_l1"], np.float32)[d])
        per_dir[d] = dict(
            wA=prep_gemm_weights(
                np.asarray(inputs["w_ih_l0"], np.float32)[d], gnw),
            biasA=prep_gemm_bias(
                np.asarray(inputs["b_ih_l0"], np.float32)[d],
                np.asarray(inputs["b_hh_l0"], np.float32)[d]),
            wD=dir_blocks(wD_nat, d).astype(bf),
            biasD=prep_gemm_bias(
                np.asarray(inputs["b_ih_l1"], np.float32)[d],
                np.asarray(inputs["b_hh_l1"], np.float32)[d]),
            gru_wT=dir_blocks(gru_wT_nat, d),
            wS0=prep_scan_weights(
                np.asarray(inputs["w_hh_l0"], np.float32)[d]).astype(bf),
            bhn0=prep_bhn_scan(np.asarray(inputs["b_hh_l0"], np.float32)[d]),
            wS1=prep_scan_weights(
                np.asarray(inputs["w_hh_l1"], np.float32)[d]).astype(bf),
            bhn1=prep_bhn_scan(np.asarray(inputs["b_hh_l1"], np.float32)[d]),
        )
        if V2_SCAN:
            per_dir[d]["bhn0"] = per_dir[d]["bhn0"].astype(bf)
            per_dir[d]["bhn1"] = per_dir[d]["bhn1"].astype(bf)
            per_dir[d]["ident8"] = prep_ident8().astype(bf)
            per_dir[d]["ones8"] = prep_ones8().astype(bf)

    in_maps = []
    for c in range(NCORE):
        d, q = c % 2, c // 2
        t_p = np.arange(SLOC * q, SLOC * (q + 1))
        t_src = t_p if d == 0 else (S - 1) - t_p
        x_loc = np.ascontiguousarray(
            x[:, t_src, :].transpose(1, 0, 2).reshape(SLOC * B, D))
        im = dict(per_dir[d])
        im["x_loc"] = x_loc
        im["x_locT"] = np.ascontiguousarray(x_loc.T)
        im["zeros"] = zeros
        im["zeros_bf"] = zeros_bf
        im["meta"] = np.array([[(1 - d) * D, SLOC * q * B]], np.uint32)
        im["w1T"] = w1T
        im["w3T"] = w3T
        im["w2T"] = w2T
        in_maps.append(im)
    return in_maps


def get_compiled(n_cores=NCORE):
    if "nc" not in _CACHE:
        nc = bacc.Bacc("TRN2", target_bir_lowering=False, debug=False,
                       num_devices=n_cores)
        build_program(nc)
        nc.compile()
        _CACHE["nc"] = nc
        _CACHE["n_cores"] = n_cores
    return _CACHE["nc"], _CACHE["n_cores"]


def kernel(**inputs) -> np.ndarray:
    in_maps = _host_inputs(inputs)
    nc, n_cores = get_compiled()
    res = run_bass_kernel_spmd(nc, in_maps, core_ids=list(range(n_cores)))
    y = np.empty((B, S, D), np.float32)
    t_loc = np.arange(SFFN)
    for c in range(n_cores):
        d, q = c % 2, c // 2
        yc = res.results[c]["y"].reshape(SFFN, B, D)
        t_p = SLOC * q + t_loc
        t_src = t_p if d == 0 else (S - 1) - t_p
        y[:, t_src, :] = yc.transpose(1, 0, 2)
    return y


# revision 77
# speedup vs baseline: 2.3041x; 2.0957x over previous
"""Trainium2 Bass kernel for nn_BidirectionalGRU (B=8,S=1024,D=1024), 8-core.

Pipeline: rmsnorm -> 2x bidirectional GRU -> out-proj + residual -> rmsnorm
-> SwiGLU FFN + residual.

8-core SPMD split:
  core c: dir d = c%2 (0=fwd, 1=bwd), pair q = c//2.
  Every core holds x pre-permuted into ITS scan order (bwd cores get
  time-reversed x from the host), so the device program is identical on
  all cores; only input data differs (plus partition_id-derived offsets).

  P0  norm stats + xg0 GEMM for my 256 time-steps x 8 batch rows
      -> AllGather over my direction group [[0,2,4,6],[1,3,5,7]]
  P1  L0 scan over all 1024 steps (my direction only); flush h.T to a
      local buffer (scan order) and to the pair-AllGather input
      (time-reversed); pairwise AllGather [[0,1],...] -> [f_rev | b_rev]
  P2  xg1 GEMM for my 256 steps from concat h0 (own dir local + peer
      slot, weights host-reordered [my-dir; other-dir]) -> AllGather
  P3  L1 scan (my direction) -> flush -> pairwise AllGather
  P4  proj + rmsnorm + SwiGLU FFN for my 128 steps (x_loc rows [0:1024])
  Host reassembles y slices (un-reversing bwd cores').

V2 scan inner loop: h.T stationary [128,8] per K-tile, w_hh.T streamed
from SBUF; 4 PE column groups (tile_position=(0,32j)) with gate layout
[rA zA nA rB zB nB] per group (PSUM cols [0:384] bank0 / [512:896]
bank1).  xg and b_hh_n enter PSUM via masked-identity openers; each
hidden half runs its own DVE/ACT chain, and the h.T transposes of step
t-1 are interleaved into step t's matmul stream (k-parity pipelining).
"""
import contextlib
import numpy as np

import concourse.bacc as bacc
import concourse.tile as tile
from concourse import mybir
from concourse.bass import ds
from concourse.bass_utils import run_bass_kernel_spmd
from concourse.masks import make_identity

F32 = mybir.dt.float32
F32R = mybir.dt.float32r
BF16 = mybir.dt.bfloat16
AF = mybir.ActivationFunctionType
ALU = mybir.AluOpType

B, S, D, H3, G, FFN = 8, 1024, 1024, 3072, 4, 2816
KD = D // 128                # 8
KF = FFN // 128              # 22
EPS = 1e-5
NP = 104                     # partitions spanned by grouped layout (3*32+8)
NCORE = 8
SLOC = S // 4                # 256 time-steps owned per core (xg phases)
NTL = (SLOC * B) // 128      # 16 local token tiles
SFFN = SLOC // 2             # 128 time-steps in FFN phase
NTF = (SFFN * B) // 128      # 8 ffn token tiles
D2 = 2 * D

AG_GROUPS = [[0, 2, 4, 6], [1, 3, 5, 7]]
A2A_GROUPS = [[0, 1], [2, 3], [4, 5], [6, 7]]

# When True, hardware For_i loops are python-unrolled so TimelineSim's
# no_exec mode can schedule the program (it cannot resolve loop branches).
SIM_UNROLL = False

# V2 scan: gate layout [rA zA nA rB zB nB] per group; xg and b_hh_n are
# folded into PSUM by identity/ones opener matmuls; the two hidden halves
# are processed as separate DVE/ACT chains and the h.T transposes of step
# t-1 are interleaved into step t's matmul stream (software pipelining).
import os as _os
V2_SCAN = _os.environ.get("KV2", "1") != "0"


def run_loop(tc, n, body):
    if SIM_UNROLL:
        for i in range(n):
            body(i)
    else:
        with tc.For_i(0, n) as iv:
            body(iv)


# ================================================================ host prep
def gate_perm():
    idx = []
    for j in range(G):
        for blk in range(3):
            base = blk * 1024 + j * 256
            idx.extend(range(base, base + 256))
    return np.array(idx)


def gate_perm2():
    idx = []
    for j in range(G):
        for h in range(2):
            for blk in range(3):
                base = blk * 1024 + j * 256 + h * 128
                idx.extend(range(base, base + 128))
    return np.array(idx)

PERM = gate_perm2() if V2_SCAN else gate_perm()
# rz columns within each 768-col group: v1 [0:512]; v2 two [.. 256] runs
RZ_MASK = (np.arange(H3) % 384 < 256) if V2_SCAN else \
          (np.arange(H3) % 768 < 512)


def prep_scan_weights(w_hh_d):
    """[3072,1024] -> [128, KD*3072]: w[p, k*H3 + n] = w_hh_perm[n, 128k+p]."""
    wp = w_hh_d[PERM]
    wt = wp.T.reshape(KD, 128, H3).transpose(1, 0, 2)
    return np.ascontiguousarray(wt.reshape(128, KD * H3), dtype=np.float32)


def prep_gemm_weights(w_ih_d, norm_w=None):
    wp = w_ih_d[PERM]
    if norm_w is not None:
        wp = wp * norm_w[None, :]
    return np.ascontiguousarray(wp.T, dtype=np.float32)


def prep_gemm_bias(b_ih_d, b_hh_d):
    """[128,3072] broadcast: rz cols get b_ih+b_hh, n cols b_ih only."""
    bi = b_ih_d[PERM].copy()
    bh = b_hh_d[PERM]
    m = np.where(RZ_MASK, bh, 0.0)
    b = (bi + m).astype(np.float32)
    return np.ascontiguousarray(np.broadcast_to(b, (128, H3)), dtype=np.float32)


def prep_ident8():
    """[128,128] masked identity: diag 1 at rows 32j+r (r<8), else 0."""
    a = np.zeros((128, 128), np.float32)
    for j in range(G):
        for r in range(8):
            a[32 * j + r, 32 * j + r] = 1.0
    return a


def prep_ones8():
    """[128,128] band broadcast: row 32j -> out cols 32j..32j+32."""
    a = np.zeros((128, 128), np.float32)
    for j in range(G):
        a[32 * j, 32 * j:32 * j + 32] = 1.0
    return a


def prep_bhn_scan(b_hh_d):
    bp = b_hh_d[PERM]
    if V2_SCAN:   # group block [rA zA nA rB zB nB] -> [nA nB]
        b6 = bp.reshape(G, 6, 128)
        bh = np.concatenate([b6[:, 2, :], b6[:, 5, :]], axis=-1)
    else:
        bh = bp.reshape(G, 3, 256)[:, 2, :]
    out = np.zeros((128, 256), np.float32)
    for j in range(G):
        out[32 * j:32 * j + 32, :] = bh[j][None, :]
    return out


# ============================================================ device builders
def build_norm_stats(tc, x_nat, s_sb, nt):
    nc = tc.nc
    with tc.tile_pool(name="nstat", bufs=3) as pool:
        for i in range(nt):
            xt = pool.tile([128, D], F32, name="xt")
            nc.sync.dma_start(xt[:], x_nat[i * 128:(i + 1) * 128, :])
            sq = pool.tile([128, D], F32, name="sq")
            ss = pool.tile([128, 1], F32, name="ss")
            nc.scalar.activation(sq[:], xt[:], AF.Square, accum_out=ss[:])
            m = pool.tile([128, 1], F32, name="m")
            nc.vector.tensor_scalar(m[:], ss[:], 1.0 / D, EPS,
                                    op0=ALU.mult, op1=ALU.add)
            r = pool.tile([128, 1], F32, name="r")
            nc.vector.reciprocal(r[:], m[:])
            nc.scalar.activation(s_sb[:, i:i + 1], r[:], AF.Sqrt)


def build_xg_gemm(tc, fetch_st, n_k, nt, w, bias, s_sb, out_v,
                  zeros_st, zrhs, wdt=F32R, U=8):
    """out[token, g, 768c] = s*(x @ w) + bias, bf16 out.

    fetch_st(stp, k, tv) -> stationary tile [128,128] for K-tile k,
    token-tile tv (tv is a register expression).
    out_v: [nt*128, G, 768] BF16 view.
    """
    nc = tc.nc
    with contextlib.ExitStack() as c:
        wp = c.enter_context(tc.tile_pool(name="xg_w", bufs=1))
        pool = c.enter_context(tc.tile_pool(name="xg_t", bufs=3))
        stp = c.enter_context(tc.tile_pool(name="xg_s", bufs=2))
        pp = c.enter_context(tc.tile_pool(name="xg_p", bufs=4, space="PSUM"))

        bias_sb = wp.tile([128, H3], F32, name="bias_sb")
        nc.sync.dma_start(bias_sb[:], bias[:, :])
        for c0 in range(0, H3, 512):
            # resident w slices for this chunk
            wc = pool.tile([128, n_k * 512], wdt, name="wc")
            for k in range(n_k):
                nc.sync.dma_start(wc[:, k * 512:(k + 1) * 512],
                                  w[k * 128:(k + 1) * 128, c0:c0 + 512])
            def chunk_body(iv, c0=c0, wc=wc):
                for u in range(U):
                    tv = iv * U + u
                    tok = tv * 128
                    sts = [fetch_st(stp, k, tv) for k in range(n_k)]
                    ps = pp.tile([128, 512], F32, name="ps")
                    nc.tensor.matmul(ps[:], zeros_st[:], zrhs[:],
                                     start=True, stop=False)
                    for k in range(n_k):
                        nc.tensor.matmul(ps[:], sts[k][:],
                                         wc[:, k * 512:(k + 1) * 512],
                                         start=False, stop=(k == n_k - 1))
                    o = pool.tile([128, 512], BF16, name="o")
                    if s_sb is not None:
                        nc.vector.scalar_tensor_tensor(
                            o[:], ps[:], s_sb[:, ds(tv, 1)],
                            bias_sb[:, c0:c0 + 512],
                            op0=ALU.mult, op1=ALU.add)
                    else:
                        nc.vector.tensor_add(o[:], ps[:],
                                             bias_sb[:, c0:c0 + 512])
                    # cols c0..c0+512 within group g0 (c0 multiple of 512;
                    # 768-group boundary: split writes)
                    cc = c0
                    while cc < c0 + 512:
                        g, gc = divmod(cc, 768)
                        take = min(768 - gc, c0 + 512 - cc)
                        nc.sync.dma_start(
                            out_v[ds(tok, 128), g, gc:gc + take],
                            o[:, cc - c0:cc - c0 + take])
                        cc += take

            run_loop(tc, nt // U, chunk_body)


def build_scan(tc, w_src, bhn_src, xg_v, hT_loc, hT_rev,
               zeros_st, zrhs, ident, zeros_bf, st2=None):
    """One GRU direction over S steps in local scan order, all B rows.

    xg_v: [S*B, G, 768] bf16 (rows t'*B + b); hT_loc: [D, S*B] bf16
    (col = t'*B + b) written in scan order; hT_rev: same shape written
    time-reversed (the AllGather payload for the pair partner).
    """
    nc = tc.nc
    with contextlib.ExitStack() as c:
        wp = c.enter_context(tc.tile_pool(name="sc_w", bufs=1))
        st = c.enter_context(tc.tile_pool(name="sc_s", bufs=1))
        pool = c.enter_context(tc.tile_pool(name="sc_t", bufs=3))
        pp = c.enter_context(tc.tile_pool(name="sc_p", bufs=2, space="PSUM"))
        ppt = c.enter_context(tc.tile_pool(name="sc_pt", bufs=2,
                                           space="PSUM"))

        w_sb = wp.tile([128, KD * H3], BF16, name="w_sb")
        nc.sync.dma_start(w_sb[:], w_src[:, :])
        bhn = wp.tile([128, 256], BF16 if V2_SCAN else F32, name="bhn")
        nc.sync.dma_start(bhn[:], bhn_src[:, :])

        hgrp = st.tile([128, 256], F32, name="hgrp")
        nc.gpsimd.memset(hgrp[:], 0.0)
        xgt_pp = None
        if V2_SCAN:
            xgt_pp = [st.tile([128, 768], BF16, name=f"xgtp{i}")
                      for i in range(2)]
            for t in xgt_pp:
                nc.sync.dma_start(t[:], zeros_bf[:, 0:768])
        # h.T history: slot u holds compact cols [c*32 + j*8 + r] (64/step)
        U = 16
        hT_hist = st.tile([128, U * 64], BF16, name="hT_hist")
        nc.sync.dma_start(hT_hist[:], zeros_bf[:, 0:U * 64])  # bf16 zeros

        # xg viewed [t, g, b, c] for per-step fetch
        xg_t = xg_v.rearrange("(t b) g c -> t g b c", b=B)

        def scan_body(iv):
            pend = []
            for u in range(U):
                t_el = iv * U + u
                if V2_SCAN:
                    gates, xgt = scan_step2_mm(
                        tc, pool, pp, st2, hT_hist, w_sb, bhn, xg_t, t_el,
                        u, (u - 1) % U, zeros_st, zrhs, pend,
                        xgt_pp[u % 2])
                    pend = [
                        scan_step2_chain(tc, pool, ppt, gates, xgt, hgrp,
                                         hT_hist, u, half, ident)
                        for half in (0, 1)]
                else:
                    scan_step(tc, pool, pp, ppt, w_sb, bhn, hgrp, hT_hist,
                              u, (u - 1) % U, xg_t, t_el, zeros_st, zrhs,
                              ident)
            if V2_SCAN:
                pend[0]()
                pend[1]()
            # flush h.T for these U steps to HBM, both orders.
            # K-tile k covers hT rows [128k, 128(k+1)) (d = 128k+p);
            # compact col in hist = (k%2)*32 + (k//2)*8 + b.
            hist3 = hT_hist.rearrange("p (s x) -> p s x", s=U)
            hl3 = hT_loc.rearrange("d (s b) -> d s b", b=B)
            hr3 = hT_rev.rearrange("d (s b) -> d s b", b=B)
            for k in range(KD):
                base = (k % 2) * 32 + (k // 2) * 8
                for b in range(B):
                    src = hist3[:, :, base + b]        # [p, slot]
                    nc.sync.dma_start(
                        hl3[k * 128:(k + 1) * 128, ds(iv * U, U), b],
                        src)
                    nc.sync.dma_start(
                        hr3[k * 128:(k + 1) * 128,
                            ds(iv * (-U) + (S - U), U), b],
                        src[:, ::-1])

        run_loop(tc, S // U, scan_body)


def scan_step2_mm(tc, pool, pp, st2, hT_hist, w_sb, bhn, xg_t, t_el,
                  slot, pslot, zeros_st, zrhs, pend, xgt):
    """V2: emit step-t matmuls with step-(t-1) transposes interleaved.

    Gate cols per group j: [rA zA nA rB zB nB] (128 each).  xg and b_hh_n
    enter PSUM via opener matmuls (identity / ones lhsT on the diagonal
    32x32 tile).  Returns the gates PSUM tile + xgt tile.
    """
    nc = tc.nc
    ident8, ones8 = st2[0], st2[1]
    for j in range(G):
        srcj = xg_t[ds(t_el, 1), j, :, :].rearrange("a b c -> (a b) c")
        nc.sync.dma_start(xgt[32 * j:32 * j + B, :], srcj)

    if pend:
        pend[0]()          # trA(t-1) + copyA(t-1)
    # PSUM layout: half-A at cols [0:384] (bank 0), half-B at [512:896]
    # (bank 1) -- a matmul output may not cross a 512-col PSUM bank.
    gates = pp.tile([128, 1024], F32, name="gates")
    # Openers (full 128-partition K=128 masked-identity matmuls): write
    # xg into the rz columns and b_hh_n into the n columns, opening each
    # bank's accumulation group at partition offset 0.
    nc.tensor.matmul(gates[:, 0:256], ident8[:], xgt[:, 0:256],
                     start=True, stop=False)
    nc.tensor.matmul(gates[:, 256:384], ones8[:], bhn[:, 0:128],
                     start=False, stop=False)
    nc.tensor.matmul(gates[:, 512:768], ident8[:], xgt[:, 384:640],
                     start=True, stop=False)
    nc.tensor.matmul(gates[:, 768:896], ones8[:], bhn[:, 128:256],
                     start=False, stop=False)

    def mm_pass(half, kpar):
        for k in range(kpar, KD, 2):
            j2, c2 = divmod(k, 2)
            lof = pslot * 64 + c2 * 32 + j2 * 8
            lhsT = hT_hist[:, lof:lof + 8]
            for j in range(G):
                wof = k * H3 + j * 768 + 384 * half
                nc.tensor.matmul(
                    gates[32 * j:32 * j + 8, 512 * half:512 * half + 384],
                    lhsT, w_sb[:, wof:wof + 384], start=False, stop=False,
                    tile_position=(0, 32 * j))

    mm_pass(0, 0)          # half-A cols, k even (needs copyA(t-1))
    if pend:
        pend[1]()          # trB(t-1) + copyB(t-1)
    mm_pass(0, 1)          # half-A cols, k odd (needs copyB(t-1))
    mm_pass(1, 0)
    mm_pass(1, 1)
    # zero-accumulate closers: stop=True over all 128 partitions closes
    # each bank's accumulation group (M=32 openers opened 32 rows/band,
    # M=8 gate matmuls could only close 8).
    nc.tensor.matmul(gates[:, 0:8], zeros_st[:], zrhs[:, 0:8],
                     start=False, stop=True)
    nc.tensor.matmul(gates[:, 512:520], zeros_st[:], zrhs[:, 0:8],
                     start=False, stop=True)
    return gates, xgt


def scan_step2_chain(tc, pool, ppt, gates, xgt, hgrp, hT_hist, slot, half,
                     ident):
    """V2 per-half DVE/ACT chain; returns the deferred transpose+copy."""
    nc = tc.nc
    hof = 512 * half           # PSUM column offset of this half
    xof = 384 * half           # xgt (SBUF) column offset
    hhof = 128 * half
    rz = pool.tile([128, 256], F32, name=f"rz{half}")
    nc.scalar.activation(rz[:NP], gates[:NP, hof:hof + 256], AF.Sigmoid)
    t2 = pool.tile([128, 128], F32, name=f"t2{half}")
    nc.vector.tensor_mul(t2[:NP], rz[:NP, 0:128],
                         gates[:NP, hof + 256:hof + 384])
    npre = pool.tile([128, 128], F32, name=f"npre{half}")
    nc.vector.tensor_add(npre[:NP], t2[:NP], xgt[:NP, xof + 256:xof + 384])
    nn = pool.tile([128, 128], F32, name=f"nn{half}")
    nc.scalar.activation(nn[:NP], npre[:NP], AF.Tanh)
    dlt = pool.tile([128, 128], F32, name=f"dlt{half}")
    nc.vector.tensor_sub(dlt[:NP], hgrp[:NP, hhof:hhof + 128], nn[:NP])
    e = pool.tile([128, 128], F32, name=f"e{half}")
    nc.vector.tensor_mul(e[:NP], rz[:NP, 128:256], dlt[:NP])
    nc.vector.tensor_add(hgrp[:NP, hhof:hhof + 128], nn[:NP], e[:NP])

    def do_tr():
        tp = ppt.tile([128, 128], F32, name=f"tp{half}")
        nc.tensor.transpose(tp[:, 0:NP], hgrp[0:NP, hhof:hhof + 128],
                            ident[0:NP, 0:NP])
        tp3 = tp.rearrange("p (j r) -> p j r", j=G)[:, :, 0:B]
        ho = hT_hist[:, slot * 64 + half * 32:slot * 64 + half * 32 + 32]
        ho3 = ho.rearrange("p (j r) -> p j r", j=G)
        nc.scalar.activation(ho3, tp3, AF.Copy)

    return do_tr


def scan_step(tc, pool, pp, ppt, w_sb, bhn, hgrp, hT_hist, slot, pslot,
              xg_t, t_el, zeros_st, zrhs, ident):
    nc = tc.nc
    xgt = pool.tile([128, 768], BF16, name="xgt")
    for j in range(G):
        srcj = xg_t[ds(t_el, 1), j, :, :].rearrange("a b c -> (a b) c")
        nc.sync.dma_start(xgt[32 * j:32 * j + B, :], srcj)

    gates = pp.tile([128, 768], F32, name="gates")
    nc.tensor.matmul(gates[:, 0:512], zeros_st[:], zrhs[:],
                     start=True, stop=False)
    nc.tensor.matmul(gates[:, 512:768], zeros_st[:], zrhs[:, 0:256],
                     start=True, stop=False)
    for k in range(KD):
        j2, c2 = divmod(k, 2)
        lof = pslot * 64 + c2 * 32 + j2 * 8
        lhsT = hT_hist[:, lof:lof + 8]
        for j in range(G):
            wof = k * H3 + j * 768
            nc.tensor.matmul(gates[32 * j:32 * j + 8, 0:512], lhsT,
                             w_sb[:, wof:wof + 512], start=False, stop=False,
                             tile_position=(0, 32 * j))
            nc.tensor.matmul(gates[32 * j:32 * j + 8, 512:768], lhsT,
                             w_sb[:, wof + 512:wof + 768], start=False,
                             stop=(k == KD - 1), tile_position=(0, 32 * j))

    grz = pool.tile([128, 512], F32, name="grz")
    nc.vector.tensor_add(grz[:NP], gates[:NP, 0:512], xgt[:NP, 0:512])
    rz = pool.tile([128, 512], F32, name="rz")
    nc.scalar.activation(rz[:NP], grz[:NP], AF.Sigmoid)
    t2a = pool.tile([128, 256], F32, name="t2a")
    nc.vector.tensor_add(t2a[:NP], gates[:NP, 512:768], bhn[:NP])
    t2 = pool.tile([128, 256], F32, name="t2")
    nc.vector.tensor_mul(t2[:NP], rz[:NP, 0:256], t2a[:NP])
    npre = pool.tile([128, 256], F32, name="npre")
    nc.vector.tensor_add(npre[:NP], t2[:NP], xgt[:NP, 512:768])
    nn = pool.tile([128, 256], F32, name="nn")
    nc.scalar.activation(nn[:NP], npre[:NP], AF.Tanh)
    dlt = pool.tile([128, 256], F32, name="dlt")
    nc.vector.tensor_sub(dlt[:NP], hgrp[:NP], nn[:NP])
    e = pool.tile([128, 256], F32, name="e")
    nc.vector.tensor_mul(e[:NP], rz[:NP, 256:512], dlt[:NP])
    nc.vector.tensor_add(hgrp[:NP], nn[:NP], e[:NP])

    tp = ppt.tile([128, 256], F32, name="tp")
    for cc in range(2):
        nc.tensor.transpose(tp[:, 128 * cc:128 * cc + NP],
                            hgrp[0:NP, 128 * cc:128 * (cc + 1)],
                            ident[0:NP, 0:NP])
    # compact copy PSUM -> hT_hist slot: col c*32 + j*8 + r  <- tp col
    # 128c + 32j + r (r<8)
    tp4 = tp.rearrange("p (c j r) -> p c j r", c=2, j=G)[:, :, :, 0:B]
    ho = hT_hist[:, slot * 64:(slot + 1) * 64]
    ho4 = ho.rearrange("p (c j r) -> p c j r", c=2, j=G)
    nc.scalar.activation(ho4, tp4, AF.Copy)


def build_proj_ffn(tc, dram, r_tcol, rd_peer_D, zeros_st, zrhs, ident):
    """P4: x2 = x + concat1 @ gru_out.T; rms; SwiGLU FFN; y."""
    nc = tc.nc
    h1l = dram["hTloc1"]
    h1p = dram["agh1_out"]
    with contextlib.ExitStack() as c:
        wp = c.enter_context(tc.tile_pool(name="pj_w", bufs=1))
        pool = c.enter_context(tc.tile_pool(name="pj_t", bufs=3))
        stp = c.enter_context(tc.tile_pool(name="pj_s", bufs=2))
        pp = c.enter_context(tc.tile_pool(name="pj_p", bufs=4, space="PSUM"))

        gw = wp.tile([128, 2 * KD * D], BF16, name="gw")
        for k in range(2 * KD):
            nc.sync.dma_start(gw[:, k * D:(k + 1) * D],
                              dram["gru_wT"][k * 128:(k + 1) * 128, :])

        def proj_body(tv):
            tok = tv * 128
            sts = []
            for k in range(2 * KD):
                stt = stp.tile([128, 128], BF16, name=f"pst{k}")
                if k < KD:
                    src = h1l[k * 128:(k + 1) * 128,
                              ds(r_tcol + tv * 128, 128)]
                else:
                    src = h1p[ds(rd_peer_D + (k - KD) * 128, 128),
                              ds(r_tcol + tv * 128, 128)]
                nc.sync.dma_start(stt[:], src)
                sts.append(stt)
            x2 = pool.tile([128, D], F32, name="x2")
            for cc in range(2):
                ps = pp.tile([128, 512], F32, name="ps")
                nc.tensor.matmul(ps[:], zeros_st[:], zrhs[:],
                                 start=True, stop=False)
                for k in range(2 * KD):
                    nc.tensor.matmul(
                        ps[:], sts[k][:],
                        gw[:, k * D + 512 * cc:k * D + 512 * cc + 512],
                        start=False, stop=(k == 2 * KD - 1))
                xt = pool.tile([128, 512], F32, name="xt")
                nc.sync.dma_start(
                    xt[:], dram["x_loc"][ds(tok, 128),
                                         512 * cc:512 * cc + 512])
                nc.vector.tensor_add(x2[:, 512 * cc:512 * cc + 512],
                                     ps[:], xt[:])
            nc.sync.dma_start(dram["x2"][ds(tok, 128), :], x2[:])
            # rms scale
            sq = pool.tile([128, D], F32, name="sq")
            ssum = pool.tile([128, 1], F32, name="ssum")
            nc.scalar.activation(sq[:], x2[:], AF.Square,
                                 accum_out=ssum[:])
            m = pool.tile([128, 1], F32, name="m")
            nc.vector.tensor_scalar(m[:], ssum[:], 1.0 / D, EPS,
                                    op0=ALU.mult, op1=ALU.add)
            r = pool.tile([128, 1], F32, name="r")
            nc.vector.reciprocal(r[:], m[:])
            s2 = pool.tile([128, 1], F32, name="s2")
            nc.scalar.activation(s2[:], r[:], AF.Sqrt)
            x2n = pool.tile([128, D], F32, name="x2n")
            nc.vector.tensor_scalar_mul(x2n[:], x2[:], s2[:])
            for k in range(KD):
                tpp = pp.tile([128, 128], F32, name="tpp")
                nc.tensor.transpose(tpp[:], x2n[:, k * 128:(k + 1) * 128],
                                    ident[:])
                xc = pool.tile([128, 128], F32R, name="xc")
                nc.scalar.activation(xc[:], tpp[:], AF.Copy)
                nc.sync.dma_start(
                    dram["x2nT"][k * 128:(k + 1) * 128, ds(tok, 128)],
                    xc[:])

        run_loop(tc, NTF, proj_body)


def build_ffn13(tc, dram, zeros_st, zrhs, ident):
    """h1 = silu(x2n@w1.T)*(x2n@w3.T); h1T -> HBM."""
    nc = tc.nc
    with contextlib.ExitStack() as c:
        wp = c.enter_context(tc.tile_pool(name="fb_w", bufs=1))
        pool = c.enter_context(tc.tile_pool(name="fb_t", bufs=3))
        stp = c.enter_context(tc.tile_pool(name="fb_s", bufs=2))
        pp = c.enter_context(tc.tile_pool(name="fb_p", bufs=2, space="PSUM"))

        w1 = wp.tile([128, KD * FFN], F32R, name="w1")
        w3 = wp.tile([128, KD * FFN], F32R, name="w3")
        for k in range(KD):
            nc.sync.dma_start(w1[:, k * FFN:(k + 1) * FFN],
                              dram["w1T"][k * 128:(k + 1) * 128, :])
            nc.sync.dma_start(w3[:, k * FFN:(k + 1) * FFN],
                              dram["w3T"][k * 128:(k + 1) * 128, :])

        FCH = [(c0, min(512, FFN - c0)) for c0 in range(0, FFN, 512)]

        def f13_body(tv):
            tok = tv * 128
            sts = []
            for k in range(KD):
                stt = stp.tile([128, 128], F32R, name=f"bst{k}")
                nc.sync.dma_start(
                    stt[:], dram["x2nT"][k * 128:(k + 1) * 128, ds(tok, 128)])
                sts.append(stt)
            for (c0, cn) in FCH:
                p1 = pp.tile([128, 512], F32, name="p1")
                p3 = pp.tile([128, 512], F32, name="p3")
                nc.tensor.matmul(p1[:, :cn], zeros_st[:], zrhs[:, :cn],
                                 start=True, stop=False)
                nc.tensor.matmul(p3[:, :cn], zeros_st[:], zrhs[:, :cn],
                                 start=True, stop=False)
                for k in range(KD):
                    nc.tensor.matmul(p1[:, :cn], sts[k][:],
                                     w1[:, k * FFN + c0:k * FFN + c0 + cn],
                                     start=False, stop=(k == KD - 1))
                    nc.tensor.matmul(p3[:, :cn], sts[k][:],
                                     w3[:, k * FFN + c0:k * FFN + c0 + cn],
                                     start=False, stop=(k == KD - 1))
                sl = pool.tile([128, 512], F32, name="sl")
                nc.scalar.activation(sl[:, :cn], p1[:, :cn], AF.Silu)
                h1c = pool.tile([128, 512], F32, name="h1c")
                nc.vector.tensor_mul(h1c[:, :cn], sl[:, :cn], p3[:, :cn])
                # transpose 128-col blocks -> h1T
                for q in range(cn // 128):
                    tpp = pp.tile([128, 128], F32, name="tpp")
                    nc.tensor.transpose(
                        tpp[:], h1c[:, q * 128:(q + 1) * 128], ident[:])
                    hc = pool.tile([128, 128], F32R, name="hc")
                    nc.scalar.activation(hc[:], tpp[:], AF.Copy)
                    kf = (c0 + q * 128) // 128
                    nc.sync.dma_start(
                        dram["h1T"][kf * 128:(kf + 1) * 128, ds(tok, 128)],
                        hc[:])

        run_loop(tc, NTF, f13_body)


def build_ffn2(tc, dram, zeros_st, zrhs):
    """y = x2 + h1 @ w2.T."""
    nc = tc.nc
    with contextlib.ExitStack() as c:
        wp = c.enter_context(tc.tile_pool(name="fc_w", bufs=1))
        pool = c.enter_context(tc.tile_pool(name="fc_t", bufs=3))
        stp = c.enter_context(tc.tile_pool(name="fc_s", bufs=2))
        pp = c.enter_context(tc.tile_pool(name="fc_p", bufs=4, space="PSUM"))

        w2 = wp.tile([128, KF * D], F32R, name="w2")
        for k in range(KF):
            nc.sync.dma_start(w2[:, k * D:(k + 1) * D],
                              dram["w2T"][k * 128:(k + 1) * 128, :])

        def f2_body(tv):
            tok = tv * 128
            sts = []
            for k in range(KF):
                stt = stp.tile([128, 128], F32R, name=f"cst{k}")
                nc.sync.dma_start(
                    stt[:],
                    dram["h1T"][k * 128:(k + 1) * 128, ds(tok, 128)])
                sts.append(stt)
            for cc in range(2):
                ps = pp.tile([128, 512], F32, name="ps")
                nc.tensor.matmul(ps[:], zeros_st[:], zrhs[:],
                                 start=True, stop=False)
                for k in range(KF):
                    nc.tensor.matmul(
                        ps[:], sts[k][:],
                        w2[:, k * D + 512 * cc:k * D + 512 * cc + 512],
                        start=False, stop=(k == KF - 1))
                xt = pool.tile([128, 512], F32, name="xt")
                nc.sync.dma_start(
                    xt[:], dram["x2"][ds(tok, 128),
                                      512 * cc:512 * cc + 512])
                yo = pool.tile([128, 512], F32, name="yo")
                nc.vector.tensor_add(yo[:], ps[:], xt[:])
                nc.sync.dma_start(
                    dram["y"][ds(tok, 128), 512 * cc:512 * cc + 512],
                    yo[:])

        run_loop(tc, NTF, f2_body)


def build_program(nc):
    dram = {}

    def din(name, shape, dt=F32R):
        dram[name] = nc.dram_tensor(name, shape, dt, kind="ExternalInput").ap()

    def dout(name, shape, dt=F32):
        dram[name] = nc.dram_tensor(name, shape, dt,
                                    kind="ExternalOutput").ap()

    def dtmp(name, shape, dt):
        dram[name] = nc.dram_tensor(name, shape, dt).ap()

    din("x_loc", [SLOC * B, D], F32)
    din("x_locT", [D, SLOC * B])
    din("wA", [D, H3])
    din("biasA", [128, H3], F32)
    din("wD", [D2, H3], BF16)
    din("biasD", [128, H3], F32)
    for L in (0, 1):
        din(f"wS{L}", [128, KD * H3], BF16)
        din(f"bhn{L}", [128, 256], BF16 if V2_SCAN else F32)
    if V2_SCAN:
        din("ident8", [128, 128], BF16)
        din("ones8", [128, 128], BF16)
    din("zeros", [128, 1024])
    din("zeros_bf", [128, 1024], BF16)
    din("meta", [1, 2], mybir.dt.uint32)
    din("gru_wT", [D2, D], BF16)
    din("w1T", [D, FFN])
    din("w3T", [D, FFN])
    din("w2T", [FFN, D])
    dout("y", [SFFN * B, D])

    for L in (0, 1):
        dtmp(f"ag{L}_in", [SLOC * B, G, 768], BF16)
        dtmp(f"ag{L}_out", [S * B, G, 768], BF16)
        dtmp(f"hTloc{L}", [D, S * B], BF16)
        dtmp(f"agh{L}_in", [D, S * B], BF16)
        dtmp(f"agh{L}_out", [D2, S * B], BF16)
    dtmp("x2", [SFFN * B, D], F32)
    dtmp("x2nT", [D, SFFN * B], F32R)
    dtmp("h1T", [FFN, SFFN * B], F32R)

    with tile.TileContext(nc) as tc:
        # per-core offsets from the meta input (nc.partition_id() breaks
        # execution in this environment): [0] = peer_slot * D (row offset
        # of the pair partner's shard in agh*_out), [1] = r_trow * B (col
        # offset of my t'-rows in the [D, S*B] h.T layout).
        t0 = nc.alloc_registers(f"meta_hp_{nc.next_id()}", mybir.ALL_ENGINES)
        nc.regs_load(t0, dram["meta"][0:1, 0:1])
        rd_peer_D = nc.snap(t0, donate=True, min_val=0, max_val=D)
        t1 = nc.alloc_registers(f"meta_tc_{nc.next_id()}", mybir.ALL_ENGINES)
        nc.regs_load(t1, dram["meta"][0:1, 1:2])
        r_tcol = nc.snap(t1, donate=True, min_val=0,
                         max_val=(S - SLOC) * B)

        with tc.tile_pool(name="consts", bufs=1) as consts:
            zeros_st = consts.tile([1, 128], F32R, name="zeros_st")
            nc.sync.dma_start(zeros_st[:], dram["zeros"][0:1, 0:128])
            zrhs = consts.tile([1, 512], F32R, name="zrhs")
            nc.sync.dma_start(zrhs[:], dram["zeros"][0:1, 0:512])
            ident = consts.tile([128, 128], F32, name="ident")
            make_identity(nc, ident[:])
            s_sb = consts.tile([128, NTL], F32, name="s_sb")
            st2 = None
            if V2_SCAN:
                i8 = consts.tile([128, 128], BF16, name="i8")
                nc.sync.dma_start(i8[:], dram["ident8"][:, :])
                o8 = consts.tile([128, 128], BF16, name="o8")
                nc.sync.dma_start(o8[:], dram["ones8"][:, :])
                st2 = (i8, o8)

            with nc.named_scope("P0_xg0"):
                build_norm_stats(tc, dram["x_loc"], s_sb, NTL)

                def fetch_x(stp, k, tv):
                    stt = stp.tile([128, 128], F32R, name=f"st{k}")
                    nc.sync.dma_start(
                        stt[:],
                        dram["x_locT"][k * 128:(k + 1) * 128, ds(tv * 128, 128)])
                    return stt

                build_xg_gemm(tc, fetch_x, KD, NTL, dram["wA"],
                              dram["biasA"], s_sb, dram["ag0_in"],
                              zeros_st, zrhs)
                nc.gpsimd.collective_compute(
                    "AllGather", ALU.bypass, replica_groups=AG_GROUPS,
                    ins=[dram["ag0_in"]], outs=[dram["ag0_out"]])

            with nc.named_scope("P1_scan0"):
                build_scan(tc, dram["wS0"], dram["bhn0"],
                           dram["ag0_out"],
                           dram["hTloc0"], dram["agh0_in"],
                           zeros_st, zrhs, ident, dram["zeros_bf"], st2)
                nc.gpsimd.collective_compute(
                    "AllGather", ALU.bypass, replica_groups=A2A_GROUPS,
                    ins=[dram["agh0_in"]], outs=[dram["agh0_out"]])

            with nc.named_scope("P2_xg1"):
                h0l = dram["hTloc0"]
                h0p = dram["agh0_out"]

                def fetch_h0(stp, k, tv):
                    stt = stp.tile([128, 128], BF16, name=f"st{k}")
                    if k < KD:
                        src = h0l[k * 128:(k + 1) * 128,
                                  ds(r_tcol + tv * 128, 128)]
                    else:
                        src = h0p[ds(rd_peer_D + (k - KD) * 128, 128),
                                  ds(r_tcol + tv * 128, 128)]
                    nc.sync.dma_start(stt[:], src)
                    return stt

                build_xg_gemm(tc, fetch_h0, 2 * KD, NTL, dram["wD"],
                              dram["biasD"], None, dram["ag1_in"],
                              zeros_st, zrhs, wdt=BF16)
                nc.gpsimd.collective_compute(
                    "AllGather", ALU.bypass, replica_groups=AG_GROUPS,
                    ins=[dram["ag1_in"]], outs=[dram["ag1_out"]])

            with nc.named_scope("P3_scan1"):
                build_scan(tc, dram["wS1"], dram["bhn1"],
                           dram["ag1_out"],
                           dram["hTloc1"], dram["agh1_in"],
                           zeros_st, zrhs, ident, dram["zeros_bf"], st2)
                nc.gpsimd.collective_compute(
                    "AllGather", ALU.bypass, replica_groups=A2A_GROUPS,
                    ins=[dram["agh1_in"]], outs=[dram["agh1_out"]])

            with nc.named_scope("P4_ffn"):
                build_proj_ffn(tc, dram, r_tcol, rd_peer_D,
                               zeros_st, zrhs, ident)
                build_ffn13(tc, dram, zeros_st, zrhs, ident)
                build_ffn2(tc, dram, zeros_st, zrhs)
            if _os.environ.get("KDEBUG"):
                for nm, shp, dt in (
                        ("ag0_in", [SLOC * B, G * 768], BF16),
                        ("hTloc0", [D, S * B], BF16),
                        ("agh0_out", [D2, S * B], BF16),
                        ("ag1_in", [SLOC * B, G * 768], BF16),
                        ("hTloc1", [D, S * B], BF16),
                        ("x2", [SFFN * B, D], F32)):
                    dbg = nc.dram_tensor("dbg_" + nm, shp, dt,
                                         kind="ExternalOutput").ap()
                    srcv = dram[nm]
                    flat = srcv.rearrange("a b c -> a (b c)") if len(
                        srcv.shape) == 3 else srcv
                    nc.sync.dma_start(dbg[:, :], flat[:, :])
    return dram


# ================================================================== driver
_CACHE = {}


def _host_inputs(inputs):
    """Build the 8 per-core input maps."""
    import ml_dtypes
    bf = ml_dtypes.bfloat16
    x = np.asarray(inputs["x"], np.float32)
    gnw = np.asarray(inputs["gru_norm_w"], np.float32)
    fnw = np.asarray(inputs["ffn_norm_w"], np.float32)

    zeros = np.zeros((128, 1024), np.float32)
    zeros_bf = np.zeros((128, 1024), bf)
    gru_wT_nat = np.ascontiguousarray(
        np.asarray(inputs["gru_out_w"], np.float32).T).astype(bf)
    w1T = np.ascontiguousarray(
        (np.asarray(inputs["w1"], np.float32) * fnw[None, :]).T)
    w3T = np.ascontiguousarray(
        (np.asarray(inputs["w3"], np.float32) * fnw[None, :]).T)
    w2T = np.ascontiguousarray(np.asarray(inputs["w2"], np.float32).T)

    def dir_blocks(w, d):
        """Reorder rows [f(0:D); b(D:2D)] -> [dir d block; dir 1-d block]."""
        if d == 0:
            return w
        return np.ascontiguousarray(np.concatenate([w[D:2 * D], w[0:D]]))

    per_dir = {}
    for d in (0, 1):
        wD_nat = prep_gemm_weights(
            np.asarray(inputs["w_ih_l1"], np.float32)[d])
        per_dir[d] = dict(
            wA=prep_gemm_weights(
                np.asarray(inputs["w_ih_l0"], np.float32)[d], gnw),
            biasA=prep_gemm_bias(
                np.asarray(inputs["b_ih_l0"], np.float32)[d],
                np.asarray(inputs["b_hh_l0"], np.float32)[d]),
            wD=dir_blocks(wD_nat, d).astype(bf),
            biasD=prep_gemm_bias(
                np.asarray(inputs["b_ih_l1"], np.float32)[d],
                np.asarray(inputs["b_hh_l1"], np.float32)[d]),
            gru_wT=dir_blocks(gru_wT_nat, d),
            wS0=prep_scan_weights(
                np.asarray(inputs["w_hh_l0"], np.float32)[d]).astype(bf),
            bhn0=prep_bhn_scan(np.asarray(inputs["b_hh_l0"], np.float32)[d]),
            wS1=prep_scan_weights(
                np.asarray(inputs["w_hh_l1"], np.float32)[d]).astype(bf),
            bhn1=prep_bhn_scan(np.asarray(inputs["b_hh_l1"], np.float32)[d]),
        )
        if V2_SCAN:
            per_dir[d]["bhn0"] = per_dir[d]["bhn0"].astype(bf)
            per_dir[d]["bhn1"] = per_dir[d]["bhn1"].astype(bf)
            per_dir[d]["ident8"] = prep_ident8().astype(bf)
            per_dir[d]["ones8"] = prep_ones8().astype(bf)

    in_maps = []
    for c in range(NCORE):
        d, q = c % 2, c // 2
        t_p = np.arange(SLOC * q, SLOC * (q + 1))
        t_src = t_p if d == 0 else (S - 1) - t_p
        x_loc = np.ascontiguousarray(
            x[:, t_src, :].transpose(1, 0, 2).reshape(SLOC * B, D))
        im = dict(per_dir[d])
        im["x_loc"] = x_loc
        im["x_locT"] = np.ascontiguousarray(x_loc.T)
        im["zeros"] = zeros
        im["zeros_bf"] = zeros_bf
        im["meta"] = np.array([[(1 - d) * D, SLOC * q * B]], np.uint32)
        im["w1T"] = w1T
        im["w3T"] = w3T
        im["w2T"] = w2T
        in_maps.append(im)
    return in_maps


def get_compiled(n_cores=NCORE):
    if "nc" not in _CACHE:
        nc = bacc.Bacc("TRN2", target_bir_lowering=False, debug=False,
                       num_devices=n_cores)
        build_program(nc)
        nc.compile()
        _CACHE["nc"] = nc
        _CACHE["n_cores"] = n_cores
    return _CACHE["nc"], _CACHE["n_cores"]


def kernel(**inputs) -> np.ndarray:
    in_maps = _host_inputs(inputs)
    nc, n_cores = get_compiled()
    res = run_bass_kernel_spmd(nc, in_maps, core_ids=list(range(n_cores)))
    y = np.empty((B, S, D), np.float32)
    t_loc = np.arange(SFFN)
    for c in range(n_cores):
        d, q = c % 2, c // 2
        yc = res.results[c]["y"].reshape(SFFN, B, D)
        t_p = SLOC * q + t_loc
        t_src = t_p if d == 0 else (S - 1) - t_p
        y[:, t_src, :] = yc.transpose(1, 0, 2)
    return y
